# revision 1
# baseline (speedup 1.0000x reference)
"""AttentionBlock (GroupNorm -> QKV -> full attention -> out-proj + residual)
for B=4, C=128, N=4096 on 8 Trainium2 NeuronCores.

Sharding: 8 cores = 4 batches x 2 query-slabs of N/2. Every core runs the
same program; the host rolls each core's x so its query slab is always
columns [0, N/2).

Key moves:
- q/k are never materialized: scores = h^T (w_q^T w_k) h + h^T (w_k^T b_q)
  with the weight product composed on the host, and the k-bias dropped
  (softmax is invariant to per-query constants). One slab projection
  qt = M^T h + bqt feeds all QK matmuls with h itself as the stationary side.
- Matmuls run in float32r (fp32 data, PE rounds to ~tf32 at full speed);
  exp'd probabilities are stored fp8e4m3 and the PV matmul contracts two
  128-key tiles per instruction with fp8 DoubleRow (2x PE throughput).
- Scores are computed transposed [j, i] so exp feeds PV with no transposes;
  softmax row sums come from all-ones matmuls over DVE-pairsummed P tiles
  accumulated in PSUM next to PV, normalized at the end of each pass by
  reciprocal_approx_fast + one multiply.
- PE executes in program order, so PV/rowsum work for a tile pair is emitted
  one tile late, keeping PE busy while ACT runs exp (software pipelining).
- The v projection absorbs the GroupNorm affine (vT = xB^T (a o w_v), with the
  shift folded through attention into the output bias), so the vT pipeline
  runs off raw bf16 x during the stats phase instead of waiting for h.
End-to-end relative error vs the fp32 reference is ~2e-4 (fp8-dominated);
cost-model (TimelineSim) per-core time ~94us.
"""

import math
import sys

if "/opt/trn_rl_repo" not in sys.path:
    sys.path.insert(0, "/opt/trn_rl_repo")

import numpy as np

C = 128
G = 8
GS = C // G  # channels per group
EPS = 1e-5
N_CORES = 8


def build(N=4096, repeat=1):
    """Build the per-core Bass program. Returns the compiled Bacc module."""
    import concourse.bacc as bacc
    import concourse.bass as bass
    import concourse.mybir as mybir
    import concourse.tile as tile

    f32 = mybir.dt.float32
    f32r = mybir.dt.float32r
    AF = mybir.ActivationFunctionType
    OP = mybir.AluOpType

    S = N // 2           # query slab width per core
    ICW = min(1024, S)   # i-chunk width (one PV/rowsum accumulation pass)
    NIC = S // ICW       # number of i-chunk passes
    NJT = N // 128       # number of j (key) tiles
    BNC = min(512, N)    # bn_stats chunk
    NBN = N // BNC
    PCW = min(512, S)    # projection/epilogue chunk width for slab-sized tensors
    NPC = S // PCW
    SCALE = 1.0 / math.sqrt(C)

    nc = bacc.Bacc("TRN2", target_bir_lowering=False, debug=False)

    x_d = nc.dram_tensor("x", [C, N], f32, kind="ExternalInput").ap()
    w_d = nc.dram_tensor("wcat", [C, 4 * C], f32, kind="ExternalInput").ap()
    m_d = nc.dram_tensor("gmask", [C, C], f32, kind="ExternalInput").ap()
    b_d = nc.dram_tensor("bcat", [C, 5], f32, kind="ExternalInput").ap()
    o_d = nc.dram_tensor("out", [C, S], f32, kind="ExternalOutput").ap()

    with tile.TileContext(nc) as tc:
        with tc.tile_pool(name="consts", bufs=1) as cp, \
             tc.tile_pool(name="big", bufs=1) as bp, \
             tc.tile_pool(name="small", bufs=3) as sp_, \
             tc.tile_pool(name="pP", bufs=6) as pP:
            _loop = tc.For_i(0, repeat, 1) if repeat > 1 else None
            if _loop is not None:
                _loop.__enter__()

            # ---- loads + constants ----
            bf16 = mybir.dt.bfloat16
            xS = bp.tile([C, N], f32, tag="x")
            for dc in range(NBN):
                nc.sync.dma_start(xS[:, dc * BNC:(dc + 1) * BNC],
                                  x_d[:, dc * BNC:(dc + 1) * BNC])
            xB = bp.tile([C, N], bf16, tag="xB")
            for dc in range(NBN):
                nc.gpsimd.tensor_copy(out=xB[:, dc * BNC:(dc + 1) * BNC],
                                      in_=xS[:, dc * BNC:(dc + 1) * BNC])
            wS = cp.tile([C, 4 * C], f32, tag="w")
            nc.sync.dma_start(wS[:], w_d[:])
            wR = cp.tile([C, 4 * C], f32r, tag="wr")
            nc.vector.tensor_copy(wR[:], wS[:])
            mS = cp.tile([C, C], f32, tag="gmask")
            nc.sync.dma_start(mS[:], m_d[:])
            bS = cp.tile([C, 5], f32, tag="bcat")
            nc.sync.dma_start(bS[:], b_d[:])
            onesS = cp.tile([C, C], f32, tag="ones")
            nc.vector.memset(onesS[:], 1.0)
            onesR = cp.tile([C, C], f32r, tag="onesr")
            nc.vector.tensor_copy(onesR[:], onesS[:])
            epsT = cp.tile([C, 1], f32, tag="eps")
            nc.vector.memset(epsT[:], EPS)
            f8 = mybir.dt.float8e4
            onesF8 = cp.tile([C, C], f8, tag="onesf8")
            nc.vector.tensor_copy(onesF8[:], onesS[:])

            hR = bp.tile([C, N], f32r, tag="h")
            qtR = bp.tile([C, S], f32r, tag="qt")
            vTR = bp.tile([C, N], f8, tag="vT")
            h2nR = bp.tile([C, S], f32r, tag="h2n")
            outS = bp.tile([C, S], f32, tag="outS")

            with tc.tile_pool(name="ps_pre", bufs=2, space="PSUM") as pre, \
                 tc.tile_pool(name="ps_vt", bufs=2, space="PSUM") as pvt:
                # ---- GroupNorm stats ----
                st6 = sp_.tile([C, NBN, 6], f32, tag="st6")
                for i in range(NBN):
                    nc.vector.bn_stats(out=st6[:, i, :], in_=xS[:, i * BNC:(i + 1) * BNC])
                mv = sp_.tile([C, 2], f32, tag="mv")
                nc.vector.bn_aggr(out=mv[:], in_=st6[:])
                # mv col1 <- mean^2 + var = E[x^2] (in place)
                nc.vector.scalar_tensor_tensor(out=mv[:, 1:2], in0=mv[:, 0:1],
                                               scalar=mv[:, 0:1], in1=mv[:, 1:2],
                                               op0=OP.mult, op1=OP.add)
                # cross-partition group reduce: gstats[c,:] = [gmean, gEx2] of c's group
                gps = pre.tile([C, 2], f32, tag="gstats")
                nc.tensor.matmul(gps[:], mS[:], mv[:], start=True, stop=True)
                gst = sp_.tile([C, 2], f32, tag="gst")
                nc.vector.tensor_copy(gst[:], gps[:])
                # xv = eps + gEx2 - gmean^2  (group variance + eps)
                i32 = mybir.dt.int32
                gv = sp_.tile([C, 1], f32, tag="gv")
                nc.vector.scalar_tensor_tensor(out=gv[:], in0=gst[:, 0:1],
                                               scalar=gst[:, 0:1], in1=gst[:, 1:2],
                                               op0=OP.mult, op1=OP.subtract)
                xv = sp_.tile([C, 1], f32, tag="xv")
                nc.vector.tensor_tensor(out=xv[:], in0=epsT[:], in1=gv[:], op=OP.subtract)
                magicT = cp.tile([C, 1], i32, tag="magic")
                nc.vector.memset(magicT[:], 0x5F3759DF)
                yh = sp_.tile([C, 1], i32, tag="yh")
                nc.vector.tensor_scalar(out=yh[:], in0=xv[:].bitcast(i32), scalar1=1,
                                        scalar2=None, op0=OP.logical_shift_right)
                nc.vector.tensor_tensor(out=yh[:], in0=magicT[:], in1=yh[:], op=OP.subtract)
                inv = sp_.tile([C, 1], f32, tag="inv")
                nc.vector.tensor_copy(inv[:], yh[:].bitcast(f32))
                tN = sp_.tile([C, 1], f32, tag="tN")
                for _ in range(2):
                    nc.vector.tensor_tensor(out=tN[:], in0=inv[:], in1=inv[:], op=OP.mult)
                    nc.vector.tensor_tensor(out=tN[:], in0=tN[:], in1=xv[:], op=OP.mult)
                    nc.vector.tensor_scalar(out=tN[:], in0=tN[:], scalar1=-0.5,
                                            scalar2=1.5, op0=OP.mult, op1=OP.add)
                    nc.vector.tensor_tensor(out=inv[:], in0=inv[:], in1=tN[:], op=OP.mult)
                aT = sp_.tile([C, 1], f32, tag="aT")
                nc.vector.tensor_tensor(out=aT[:], in0=bS[:, 3:4], in1=inv[:], op=OP.mult)
                bT = sp_.tile([C, 1], f32, tag="bT")
                nc.vector.tensor_tensor(out=bT[:], in0=gst[:, 0:1], in1=aT[:], op=OP.mult)
                nc.vector.tensor_tensor(out=bT[:], in0=bS[:, 4:5], in1=bT[:], op=OP.subtract)
                # v absorbs the GN affine: v = (a.w_v)^T x + w_v^T b; the
                # constant term rides through attention as a per-channel
                # offset on h2n and folds into the output bias via W_out.
                wvAB = cp.tile([C, C], bf16, tag="wvAB")
                nc.vector.tensor_scalar(out=wvAB[:], in0=wS[:, 2 * C:3 * C],
                                        scalar1=aT[:], scalar2=None, op0=OP.mult)
                vc_p = pre.tile([C, 2], f32, tag="gstats", name="vc_p")
                nc.tensor.matmul(vc_p[:, 0:1], wS[:, 2 * C:3 * C], bT[:],
                                 start=True, stop=True)
                vcS = sp_.tile([C, 1], f32, tag="vcS")
                nc.vector.tensor_copy(vcS[:], vc_p[:, 0:1])
                wov_p = pre.tile([C, 2], f32, tag="gstats", name="wov_p")
                nc.tensor.matmul(wov_p[:, 0:1], wS[:, 3 * C:4 * C], vcS[:],
                                 start=True, stop=True)
                beffT = sp_.tile([C, 1], f32, tag="beffT")
                nc.vector.tensor_tensor(out=beffT[:], in0=bS[:, 2:3],
                                        in1=wov_p[:, 0:1], op=OP.add)
                # h = a*x + b (f32r); first NPC chunks feed qt, the rest only
                # feed QK weights for late tiles so they can trail the vT copies
                for c in range(NPC):
                    nc.vector.tensor_scalar(out=hR[:, c * BNC:(c + 1) * BNC],
                                            in0=xS[:, c * BNC:(c + 1) * BNC],
                                            scalar1=aT[:], scalar2=bT[:],
                                            op0=OP.mult, op1=OP.add)

                # ---- projections ----
                for c in range(NPC):
                    # qt = (w_q^T w_k)^T h + w_k^T b_q;  scores = h^T qt
                    qtp = pre.tile([C, PCW], f32, tag="qtp")
                    nc.tensor.matmul(qtp[:], wR[:, 0:C], hR[:, PCW * c:PCW * (c + 1)],
                                     start=True, stop=True)
                    nc.scalar.activation(out=qtR[:, PCW * c:PCW * (c + 1)], in_=qtp[:],
                                         func=AF.Identity, bias=bS[:, 0:1], scale=1.0)
                VTW = min(1024, N)
                for g in range(N // VTW):
                    vtp = pvt.tile([C, VTW], f32, tag="vtp")
                    for jj in range(VTW // 128):
                        tj = (VTW // 128) * g + jj
                        nc.tensor.matmul(vtp[:, 128 * jj:128 * (jj + 1)],
                                         xB[:, 128 * tj:128 * (tj + 1)], wvAB[:],
                                         start=True, stop=True)
                    nc.vector.tensor_copy(vTR[:, VTW * g:VTW * (g + 1)], vtp[:])
                for c in range(NPC, N // BNC):
                    nc.vector.tensor_scalar(out=hR[:, c * BNC:(c + 1) * BNC],
                                            in0=xS[:, c * BNC:(c + 1) * BNC],
                                            scalar1=aT[:], scalar2=bT[:],
                                            op0=OP.mult, op1=OP.add)


            # ---- attention ----
            # sT double-buffered at STW wide (one exp op per tile). PE runs in
            # program order, so PV/ones for tile t are emitted one tile late:
            # while exp(t) runs on ACT, PE issues QK(t+1) instead of stalling.
            STW = min(1024, ICW)   # sT tile / exp chunk width
            NST = ICW // STW
            MMW = min(512, STW)    # matmul free-dim chunk
            NMM = STW // MMW
            NPAIR = NJT // 2  # rowsum matmuls run on pairwise P-sums (DVE adds)
            with tc.tile_pool(name="ps_sT", bufs=2, space="PSUM") as psT, \
                 tc.tile_pool(name="ps_rs", bufs=1, space="PSUM") as prs, \
                 tc.tile_pool(name="ps_h2", bufs=1, space="PSUM") as ph2:
                acc = {}        # ic -> (h2p, rsp)
                pend_pv = None  # (ic, odd t, Ppair) awaiting PV emission
                pend_ones = None  # (ic, pair_idx, Ps2) awaiting ones-MM emission

                def emit_pv(p):
                    # fp8 DoubleRow: one matmul contracts the pair of j-tiles
                    # (tp-1, tp); called only at odd tp.
                    icp, tp, Ppair = p
                    h2p = acc[icp][0]
                    pi = tp // 2
                    vpair = vTR[:, 256 * pi:256 * (pi + 1)].rearrange(
                        "p (two c) -> p two c", two=2)
                    for m in range(NMM):
                        nc.tensor.matmul(
                            h2p[:, m * MMW:(m + 1) * MMW], vpair,
                            Ppair[:, :, m * MMW:(m + 1) * MMW],
                            start=(pi == 0), stop=(pi == NJT // 2 - 1),
                            perf_mode=mybir.MatmulPerfMode.DoubleRow)

                def emit_ones(p, first=None, last=None, fp8=False):
                    icp, pi, Ps2p = p
                    rsp = acc[icp][1]
                    st = first if first is not None else (pi == 0)
                    sp2 = last if last is not None else False
                    lhs = onesF8[:] if fp8 else onesR[:]
                    for m in range(NMM):
                        nc.tensor.matmul(
                            rsp[:, m * MMW:(m + 1) * MMW], lhs,
                            Ps2p[:, m * MMW:(m + 1) * MMW],
                            start=st, stop=sp2)

                def finish_pass(ic):
                    h2p, rsp = acc[ic]
                    FCW = min(512, ICW)
                    for fc in range(ICW // FCW):
                        sl_i = slice(ic * ICW + fc * FCW, ic * ICW + (fc + 1) * FCW)
                        sl_f = slice(fc * FCW, (fc + 1) * FCW)
                        recipB = sp_.tile([C, FCW], f32, tag="recipB")
                        nc.vector.reciprocal_approx_fast(out=recipB[:], in_=rsp[:, sl_f])
                        nc.vector.tensor_tensor(out=h2nR[:, sl_i], in0=h2p[:, sl_f],
                                                in1=recipB[:], op=OP.mult)

                for ic in range(NIC):
                    acc[ic] = (ph2.tile([C, ICW], f32, tag="h2u", name=f"h2u{ic}"),
                               prs.tile([C, ICW], f32, tag="rs", name=f"rs{ic}"))
                    Ppair = None
                    for t in range(NJT):
                        for c2 in range(NST):
                            i0 = ic * ICW + c2 * STW
                            sT = psT.tile([C, STW], f32, tag="sT")
                            for m in range(NMM):
                                nc.tensor.matmul(
                                    sT[:, m * MMW:(m + 1) * MMW],
                                    hR[:, 128 * t:128 * (t + 1)],
                                    qtR[:, i0 + m * MMW:i0 + (m + 1) * MMW],
                                    start=True, stop=True)
                            if t % 2 == 0:
                                Ppair = pP.tile([C, 2, STW], f8, tag="P",
                                                name=f"P{ic}_{t}")
                            nc.scalar.activation(out=Ppair[:, t % 2, :], in_=sT[:],
                                                 func=AF.Exp, scale=SCALE)
                            if t % 2 == 1:
                                pend_pv = (ic, t, Ppair)
                                continue
                            if pend_pv is not None:
                                emit_pv(pend_pv)
                                tp = pend_pv[1]
                                Pp = pend_pv[2]
                                if tp == NJT - 1:
                                    # tail of the pass: direct fp8 ones-MMs so
                                    # the rowsum doesn't wait on a DVE pair-add
                                    if pend_ones is not None:
                                        emit_ones(pend_ones)
                                        pend_ones = None
                                    emit_ones((pend_pv[0], -1, Pp[:, 0, :]),
                                              first=False, last=False, fp8=True)
                                    emit_ones((pend_pv[0], -1, Pp[:, 1, :]),
                                              first=False, last=True, fp8=True)
                                    finish_pass(pend_pv[0])
                                else:
                                    # DVE pair-sum of the two P slices just used
                                    Ps2 = pP.tile([C, STW], f32r, tag="Ps2")
                                    nc.vector.tensor_tensor(
                                        out=Ps2[:], in0=Pp[:, 0, :],
                                        in1=Pp[:, 1, :], op=OP.add)
                                    if pend_ones is not None:
                                        emit_ones(pend_ones)
                                    pend_ones = (pend_pv[0], tp // 2, Ps2)
                                pend_pv = None
                emit_pv(pend_pv)
                tp = pend_pv[1]
                Pp = pend_pv[2]
                if pend_ones is not None:
                    emit_ones(pend_ones)
                    pend_ones = None
                emit_ones((pend_pv[0], -1, Pp[:, 0, :]), first=False, last=False,
                          fp8=True)
                emit_ones((pend_pv[0], -1, Pp[:, 1, :]), first=False, last=True,
                          fp8=True)
                finish_pass(pend_pv[0])

            # ---- out projection + bias + residual ----
            with tc.tile_pool(name="ps_ep", bufs=2, space="PSUM") as pep:
                for c in range(NPC):
                    pop = pep.tile([C, PCW], f32, tag="pop")
                    nc.tensor.matmul(pop[:], wR[:, 3 * C:4 * C],
                                     h2nR[:, PCW * c:PCW * (c + 1)], start=True, stop=True)
                    nc.vector.scalar_tensor_tensor(
                        out=outS[:, PCW * c:PCW * (c + 1)], in0=pop[:], scalar=beffT[:],
                        in1=xS[:, PCW * c:PCW * (c + 1)], op0=OP.add, op1=OP.add)
                    nc.sync.dma_start(o_d[:, PCW * c:PCW * (c + 1)],
                                      outS[:, PCW * c:PCW * (c + 1)])
            if _loop is not None:
                _loop.__exit__(None, None, None)

    nc.compile()
    return nc


def host_inputs(x, gn_w, gn_b, w_qkv, b_qkv, w_out, b_out):
    """Build the 8 per-core input maps from the full problem inputs."""
    x = np.asarray(x, dtype=np.float32)
    B, _, N = x.shape
    S = N // 2
    w_qkv = np.asarray(w_qkv, np.float32)
    w_out = np.asarray(w_out, np.float32)
    b_qkv = np.asarray(b_qkv, np.float32)
    b_out = np.asarray(b_out, np.float32)
    gn_w = np.asarray(gn_w, np.float32)
    gn_b = np.asarray(gn_b, np.float32)

    # scores = h^T (w_q^T w_k) h + h^T (w_k^T b_q); the k bias is
    # softmax-invariant and dropped, q/k are never materialized on device.
    M = w_qkv[0:C].T @ w_qkv[C:2 * C]
    wcat = np.concatenate(
        [M, np.zeros((C, C), np.float32), w_qkv[2 * C:3 * C].T, w_out.T],
        axis=1).astype(np.float32)   # [C, 4C]: [M, unused, w_v^T, w_out^T]
    gidx = np.arange(C) // GS
    gmask = (gidx[:, None] == gidx[None, :]).astype(np.float32) / GS
    b_eff = b_out + w_out @ b_qkv[2 * C:3 * C]
    bqt = w_qkv[C:2 * C].T @ b_qkv[0:C]
    bcat = np.stack([bqt, b_qkv[C:2 * C], b_eff, gn_w, gn_b], axis=1)
    bcat = np.ascontiguousarray(bcat, np.float32)       # [C, 5]

    in_maps = []
    for core in range(N_CORES):
        b, half = divmod(core, 2)
        xb = np.roll(x[b], -half * S, axis=1)
        in_maps.append({"x": np.ascontiguousarray(xb), "wcat": wcat,
                        "gmask": gmask, "bcat": bcat})
    return in_maps


_NC_CACHE = {}
_RUNNER_CACHE = {}


def _make_runner(nc):
    """Compile-once runner: replicates bass2jax.run_bass_via_pjrt but keeps the
    jitted sharded callable so repeat executions skip recompilation."""
    import jax
    import concourse.mybir as mybir
    from jax.sharding import Mesh, PartitionSpec
    from jax.experimental.shard_map import shard_map
    from concourse.bass2jax import (_bass_exec_p, install_neuronx_cc_hook,
                                    partition_id_tensor)

    install_neuronx_cc_hook()
    partition_name = nc.partition_id_tensor.name if nc.partition_id_tensor else None
    in_names, out_names, out_avals, zero_shapes = [], [], [], []
    for alloc in nc.m.functions[0].allocations:
        if not isinstance(alloc, mybir.MemoryLocationSet):
            continue
        name = alloc.memorylocations[0].name
        if alloc.kind == "ExternalInput":
            if name == partition_name:
                continue
            in_names.append(name)
        elif alloc.kind == "ExternalOutput":
            out_names.append(name)
            shape = tuple(alloc.tensor_shape)
            dtype = mybir.dt.np(alloc.dtype)
            out_avals.append(jax.core.ShapedArray(shape, dtype))
            zero_shapes.append((shape, dtype))
    n_params = len(in_names)
    all_names = in_names + out_names
    if partition_name is not None:
        all_names = all_names + [partition_name]
    donate = tuple(range(n_params, n_params + len(out_names)))

    def _body(*args):
        operands = list(args)
        if partition_name is not None:
            operands.append(partition_id_tensor())
        return tuple(_bass_exec_p.bind(
            *operands, out_avals=tuple(out_avals), in_names=tuple(all_names),
            out_names=tuple(out_names), lowering_input_output_aliases=(),
            sim_require_finite=True, sim_require_nnan=True, nc=nc))

    devices = jax.devices()[:N_CORES]
    mesh = Mesh(np.asarray(devices), ("core",))
    specs = (PartitionSpec("core"),)
    sharded = jax.jit(
        shard_map(_body, mesh=mesh,
                  in_specs=specs * (n_params + len(out_names)),
                  out_specs=specs * len(out_names), check_rep=False),
        donate_argnums=donate, keep_unused=True)

    def run(in_maps):
        concat_in = [np.concatenate([np.asarray(m[nm]) for m in in_maps], axis=0)
                     for nm in in_names]
        concat_zeros = [np.zeros((N_CORES * s[0], *s[1:]), d) for s, d in zero_shapes]
        out_arrs = sharded(*concat_in, *concat_zeros)
        out_arrs = [np.asarray(a) for a in out_arrs]
        return [{nm: out_arrs[i].reshape(N_CORES, *out_avals[i].shape)[c]
                 for i, nm in enumerate(out_names)} for c in range(N_CORES)]

    return run


def get_runner(N=4096):
    if N not in _RUNNER_CACHE:
        if N not in _NC_CACHE:
            _NC_CACHE[N] = build(N)
        _RUNNER_CACHE[N] = _make_runner(_NC_CACHE[N])
    return _RUNNER_CACHE[N]


def kernel(x, gn_w, gn_b, w_qkv, b_qkv, w_out, b_out):
    from concourse._compat import axon_active

    x = np.asarray(x, dtype=np.float32)
    B, _, N = x.shape
    S = N // 2
    in_maps = host_inputs(x, gn_w, gn_b, w_qkv, b_qkv, w_out, b_out)
    if axon_active():
        results = get_runner(N)(in_maps)
    else:
        from concourse.bass_utils import run_bass_kernel_spmd

        if N not in _NC_CACHE:
            _NC_CACHE[N] = build(N)
        results = run_bass_kernel_spmd(_NC_CACHE[N], in_maps,
                                       core_ids=list(range(N_CORES))).results
    out = np.empty((B, C, N), dtype=np.float32)
    for core in range(N_CORES):
        b, half = divmod(core, 2)
        out[b, :, half * S:(half + 1) * S] = results[core]["out"]
    return out



# revision 4
# speedup vs baseline: 1.1504x; 1.1504x over previous
"""AttentionBlock (GroupNorm -> QKV -> full attention -> out-proj + residual)
for B=4, C=128, N=4096 on 8 Trainium2 NeuronCores.

Sharding: 8 cores = 4 batches x 2 query-slabs of N/2. Every core runs the
same program; the host rolls each core's x so its query slab is always
columns [0, N/2).

Key moves:
- All big matmuls are fp8 DoubleRow (0.5 cyc/col): channels split as
  c = 2p + t into a [64, 2, *] layout so the C=128 contraction rides the
  256-row DoubleRow path. z = a*x (GN scale only; the shift b folds into a
  per-channel bias u = M^T b + wk^T bq added during the qtM psum->fp8
  conversion, since scores s[j,i] = z_j . (M^T z_i + u)).
- exp runs on THREE engines: ACT (real Exp), DVE and Pool (Schraudolph:
  int8(A*s + B) bitcast as fp8e4m3 - one tensor_scalar op). Each j-tile's
  scores are a [C, 512] PSUM tile; P pairs assemble in SBUF for fp8
  DoubleRow PV and rowsum (ones) matmuls - no DVE pair-adds anywhere.
- One PSUM layout for the whole kernel: a 6-slot ring of 1-bank tiles
  (scores, qtM/vT staging, out-proj) + 1 bank each for the PV and rowsum
  accumulators. No mid-kernel pool transitions.
- v comes from z via wv fp8 DoubleRow sharing QK's ldweights; GroupNorm
  stats run once in 128-layout (exact b) with a dup'd tiny Newton chain in
  the split layout for a; ACT does psum->fp8 conversions while it waits
  for z during the ramp.
- Per-pass (i-window 512) finish: recip (DVE), normalize (Pool), out-proj,
  epilogue + DMA out, pipelined against the next pass.
"""

import math
import sys

if "/opt/trn_rl_repo" not in sys.path:
    sys.path.insert(0, "/opt/trn_rl_repo")

import numpy as np

C = 128
G = 8
GS = C // G  # channels per group
EPS = 1e-5
N_CORES = 8
NP = C // 2  # 64: partition count of the split-channel layout


def build(N=4096, repeat=1, cfg=None):
    """Build the per-core Bass program. Returns the compiled Bacc module."""
    cfg = dict(cfg or {})
    P0 = cfg.get("p0", (9, 7))           # pass-0 exp PAIR counts A/D
    PS = cfg.get("ps", (10, 6))           # steady-pass exp PAIR counts
    VTE = cfg.get("vte", "ADAD")     # vT conv engines by chunk
    QTE = cfg.get("qte", "AADD")         # qtM conv engines by window
    PVLAG = cfg.get("pvlag", 5)          # pairs of PV/ones lag
    import concourse.bacc as bacc
    import concourse.bass as bass
    import concourse.mybir as mybir
    import concourse.tile as tile

    f32 = mybir.dt.float32
    f32r = mybir.dt.float32r
    f8 = mybir.dt.float8e4
    i8 = mybir.dt.int8
    i32 = mybir.dt.int32
    AF = mybir.ActivationFunctionType
    OP = mybir.AluOpType
    DR = mybir.MatmulPerfMode.DoubleRow

    S = N // 2            # query slab width per core
    IW = 512              # i-window per pass
    NPASS = S // IW       # 4
    NJT = N // 128        # 32 j tiles
    XC = 512              # xS DMA/bnstats chunk
    NXC = N // XC         # 8
    X2C = 1024            # x2x DMA / z2x chunk
    NX2 = N // X2C        # 4
    SCALE = 1.0 / math.sqrt(C)
    # Schraudolph: int8 y = trunc(A*s_raw + B) bitcast fp8e4m3 ~ exp(SCALE*s)
    SCH_A = 8.0 * math.log2(math.e) * SCALE
    SCH_B = 8.0 * (7.0 - 0.045) + 0.5

    # exp engine per (pass, jtile): A=ACT, D=DVE, P=Pool, rate-weighted
    # (ACT 0.61us, DVE 0.66, Pool 0.81 per tile) with D/P's extra per-pass
    # work (recip/normalize on D, epilogue on P, conversions in pass 0)
    # subtracted from their shares.
    def mk_assign(na, nd):
        # per-PAIR engine letters, doubled to tiles: same-engine pairs let
        # one amortized pair-op cover both halves
        out, acc = [], {"A": 0.0, "D": 0.0}
        want = {"A": na, "D": nd}
        for i in range(NJT // 2):
            e = max(want, key=lambda k: want[k] * (i + 1) / (NJT // 2) - acc[k])
            acc[e] += 1
            out += [e, e]
        return out

    ASSIGN = mk_assign(*P0)
    for _ in range(NPASS - 1):
        ASSIGN += mk_assign(*PS)

    nc = bacc.Bacc("TRN2", target_bir_lowering=False, debug=False)

    x_d = nc.dram_tensor("x", [C, N], f32, kind="ExternalInput").ap()
    # wcat cols: [Mc | wv^T | wout^T | gmask2x | gmask | bcat(5)]
    w_d = nc.dram_tensor("wcat", [C, 5 * C + 5], f32, kind="ExternalInput").ap()
    # wcat2 cols: [M2xc | wv2x | bcat2(4)]
    w2_d = nc.dram_tensor("wcat2", [NP, 4 * C + 4], f32,
                          kind="ExternalInput").ap()
    o_d = nc.dram_tensor("out", [C, S], f32, kind="ExternalOutput").ap()

    with tile.TileContext(nc) as tc:
        with tc.tile_pool(name="consts", bufs=1) as cp, \
             tc.tile_pool(name="big", bufs=1) as bp, \
             tc.tile_pool(name="small", bufs=3) as sp_, \
             tc.tile_pool(name="pP", bufs=10) as pP, \
             tc.tile_pool(name="ring", bufs=3, space="PSUM") as ring, \
             tc.tile_pool(name="ph2", bufs=1, space="PSUM") as ph2, \
             tc.tile_pool(name="prs", bufs=1, space="PSUM") as prs:
            # ---- DMAs: xS chunks first (stats chain), consts, x2x ----
            xS = bp.tile([C, N], f32, tag="x")
            for c in range(4):
                nc.sync.dma_start(xS[:, c * 2 * XC:(c + 1) * 2 * XC],
                                  x_d[:, c * 2 * XC:(c + 1) * 2 * XC])
            wS = cp.tile([C, 5 * C + 5], f32, tag="w")
            nc.sync.dma_start(wS[:], w_d[:])
            bS = wS[:, 5 * C:5 * C + 5]
            x2x = bp.tile([NP, 2, N], f32, tag="x2x")
            x2v = x_d.rearrange("(p two) n -> p two n", two=2)
            nc.sync.dma_start(x2x[:, :, 0:X2C], x2v[:, :, 0:X2C])
            wc2 = cp.tile([NP, 4 * C + 4], f32, tag="w2")
            nc.sync.dma_start(wc2[:], w2_d[:])
            b2S = wc2[:, 4 * C:4 * C + 4]
            for c in range(1, NX2):
                nc.sync.dma_start(x2x[:, :, c * X2C:(c + 1) * X2C],
                                  x2v[:, :, c * X2C:(c + 1) * X2C])

            # ---- ACT exp-table preload (runs during DMA) ----
            trash = sp_.tile([C, 1], f32, tag="trash")
            nc.vector.memset(trash[:], 0.0)
            nc.scalar.activation(out=trash[:], in_=trash[:], func=AF.Exp,
                                 scale=1.0)

            # ---- const conversions ----
            M2xc8 = cp.tile([NP, 2, C], f8, tag="m2xc8")
            nc.gpsimd.tensor_copy(M2xc8[:], wc2[:, 0:2 * C])
            wv2x8 = cp.tile([NP, 2, C], f8, tag="wv2x8")
            nc.gpsimd.tensor_copy(wv2x8[:], wc2[:, 2 * C:4 * C])
            onesP = cp.tile([C, 2 * C], f8, tag="onesp")
            nc.gpsimd.memset(onesP[:], 1.0)
            onesPv = onesP[:].rearrange("p (two c) -> p two c", two=2)
            woutR = cp.tile([C, C], f32r, tag="woutr")
            nc.gpsimd.tensor_copy(woutR[:], wS[:, 2 * C:3 * C])

            # persistent big tensors
            z2x = bp.tile([NP, 2, N], f8, tag="z2x")
            qtM8 = bp.tile([NP, 2, S], f8, tag="qtm8")
            vTR = bp.tile([C, N], f8, tag="vT")

            def rsqrt_chain(eng, shape, gmean, gex2, tag):
                """Magic-Newton rsqrt(var+eps) on [part, k] APs -> inv tile."""
                part, k = shape
                gm2 = sp_.tile([part, k], f32, tag=tag + "gm2")
                eng.tensor_tensor(out=gm2[:], in0=gmean, in1=gmean, op=OP.mult)
                xv = sp_.tile([part, k], f32, tag=tag + "xv")
                eng.tensor_tensor(out=xv[:], in0=gex2, in1=gm2[:],
                                  op=OP.subtract)
                eng.tensor_scalar(out=xv[:], in0=xv[:], scalar1=1.0,
                                  scalar2=EPS, op0=OP.mult, op1=OP.add)
                yh = sp_.tile([part, k], i32, tag=tag + "yh")
                eng.tensor_scalar(out=yh[:], in0=xv[:].bitcast(i32),
                                  scalar1=1, scalar2=None,
                                  op0=OP.logical_shift_right)
                eng.tensor_scalar(out=yh[:], in0=yh[:], scalar1=-1,
                                  scalar2=0x5F3759DF, op0=OP.mult, op1=OP.add)
                y0 = yh[:].bitcast(f32)
                tN = sp_.tile([part, k], f32, tag=tag + "tN")
                eng.tensor_tensor(out=tN[:], in0=y0, in1=y0, op=OP.mult)
                eng.tensor_tensor(out=tN[:], in0=tN[:], in1=xv[:], op=OP.mult)
                eng.tensor_scalar(out=tN[:], in0=tN[:], scalar1=-0.5,
                                  scalar2=1.5, op0=OP.mult, op1=OP.add)
                inv = sp_.tile([part, k], f32, tag=tag + "inv")
                eng.tensor_tensor(out=inv[:], in0=y0, in1=tN[:], op=OP.mult)
                return inv

            # ---- GroupNorm stats: bnstats (DVE) -> group matmuls (PE) ----
            st6 = sp_.tile([C, NXC, 6], f32, tag="st6")
            for c in range(NXC):
                nc.vector.bn_stats(out=st6[:, c, :],
                                   in_=xS[:, c * XC:(c + 1) * XC])
            mv = sp_.tile([C, 2], f32, tag="mv")
            nc.vector.bn_aggr(out=mv[:], in_=st6[:])
            nc.vector.scalar_tensor_tensor(out=mv[:, 1:2], in0=mv[:, 0:1],
                                           scalar=mv[:, 0:1], in1=mv[:, 1:2],
                                           op0=OP.mult, op1=OP.add)
            gps = ring.tile([C, 2], f32, tag="sT", name="gps")
            nc.tensor.matmul(gps[:], wS[:, 4 * C:5 * C], mv[:], start=True,
                             stop=True)
            gst = sp_.tile([C, 2], f32, tag="gst")
            nc.vector.tensor_copy(gst[:], gps[:])
            gap = ring.tile([NP, 2, 2], f32, tag="sT", name="gap")
            for t in range(2):
                nc.tensor.matmul(gap[:, t, :],
                                 wS[:, 3 * C + NP * t:3 * C + NP * (t + 1)],
                                 mv[:], start=True, stop=True)
            gas = sp_.tile([NP, 2, 2], f32, tag="gas")
            nc.vector.tensor_copy(gas[:], gap[:])

            # split layout scale a2x on DVE; exact 128-layout b on Pool
            inv2x = rsqrt_chain(nc.vector, [NP, 2], gas[:, :, 0:1],
                                gas[:, :, 1:2], "n2")
            a2x = sp_.tile([NP, 2], f32, tag="a2x")
            nc.vector.tensor_tensor(out=a2x[:], in0=b2S[:, 0:2], in1=inv2x[:],
                                    op=OP.mult)
            inv128 = rsqrt_chain(nc.vector, [C, 1], gst[:, 0:1], gst[:, 1:2],
                                 "n1")
            aT = sp_.tile([C, 1], f32, tag="aT")
            nc.vector.tensor_tensor(out=aT[:], in0=bS[:, 3:4], in1=inv128[:],
                                    op=OP.mult)
            bT = sp_.tile([C, 1], f32, tag="bT")
            nc.vector.tensor_tensor(out=bT[:], in0=gst[:, 0:1], in1=aT[:],
                                    op=OP.mult)
            nc.vector.tensor_tensor(out=bT[:], in0=bS[:, 4:5], in1=bT[:],
                                    op=OP.subtract)

            # u2x = (M^T b + bqt) in split layout; beff = bcat + w_out wv^T b
            u_p = ring.tile([NP, 2], f32, tag="sT", name="u_p")
            for t in range(2):
                nc.tensor.matmul(u_p[:, t:t + 1], wS[:, NP * t:NP * (t + 1)],
                                 bT[:], start=True, stop=True)
            u2x = sp_.tile([NP, 2], f32, tag="u2xf")
            nc.vector.tensor_tensor(out=u2x[:], in0=u_p[:], in1=b2S[:, 2:4],
                                    op=OP.add)
            vc_p = ring.tile([C, 2], f32, tag="sT", name="vc_p")
            nc.tensor.matmul(vc_p[:, 0:1], wS[:, C:2 * C], bT[:], start=True,
                             stop=True)
            vcS = sp_.tile([C, 1], f32, tag="vcS")
            nc.vector.tensor_copy(vcS[:], vc_p[:, 0:1])
            wov_p = ring.tile([C, 2], f32, tag="sT", name="wov_p")
            nc.tensor.matmul(wov_p[:, 0:1], wS[:, 2 * C:3 * C], vcS[:],
                             start=True, stop=True)
            beffT = sp_.tile([C, 1], f32, tag="beffT")
            nc.vector.tensor_tensor(out=beffT[:], in0=bS[:, 2:3],
                                    in1=wov_p[:, 0:1], op=OP.add)

            # ---- z2x = a2x * x2x -> fp8 (all on DVE: 2x-port mode there).
            # high_priority so the scheduler never defers z chunks behind
            # pass-0 exps (z gates every QK of the j-tile it covers).
            with tc.high_priority():
                for c in range(NX2):
                    sl = slice(c * X2C, (c + 1) * X2C)
                    for t in range(2):
                        nc.vector.tensor_scalar(out=z2x[:, t, sl],
                                                in0=x2x[:, t, sl],
                                                scalar1=a2x[:, t:t + 1],
                                                scalar2=None, op0=OP.mult)

            def emit_qtm(ic, conv):
                """qtM for i-window ic: 2 DR matmuls + 2 psum->fp8(+u) convs.
                conv in {'D','P','A'}."""
                sl = slice(ic * IW, (ic + 1) * IW)
                for t in range(2):
                    qp = ring.tile([NP, IW], f32, tag="sT",
                                   name=f"qp{ic}_{t}")
                    nc.tensor.matmul(qp[:], M2xc8[:, :, NP * t:NP * (t + 1)],
                                     z2x[:, :, sl], start=True, stop=True,
                                     perf_mode=DR)
                    if conv == "A":
                        nc.scalar.activation(out=qtM8[:, t, sl], in_=qp[:],
                                             func=AF.Identity,
                                             bias=u2x[:, t:t + 1], scale=1.0)
                    else:
                        nc.vector.tensor_scalar(out=qtM8[:, t, sl], in0=qp[:],
                                                scalar1=u2x[:, t:t + 1],
                                                scalar2=None, op0=OP.add)

            # qtM for pass 0 first (gates attention start); rest follow
            emit_qtm(0, QTE[0])

            def emit_exp(sT2, P2, sl2, eng):
                """exp over sT2[:, sl2, :] -> P2[:, sl2, :]; sl2 covers one
                or both pair halves (fused when both on one engine)."""
                if eng == "A":
                    nc.scalar.activation(out=P2[:, sl2, :],
                                         in_=sT2[:, sl2, :],
                                         func=AF.Exp, scale=SCALE)
                else:
                    nc.vector.tensor_scalar(
                        out=P2[:, sl2, :].bitcast(i8), in0=sT2[:, sl2, :],
                        scalar1=SCH_A, scalar2=SCH_B,
                        op0=OP.mult, op1=OP.add)

            def emit_pv_ones(h2p, rsp, pr, P2, npr):
                vpair = vTR[:, 256 * pr:256 * (pr + 1)].rearrange(
                    "p (two c) -> p two c", two=2)
                nc.tensor.matmul(h2p[:], vpair, P2[:], start=(pr == 0),
                                 stop=(pr == npr - 1), perf_mode=DR)
                nc.tensor.matmul(rsp[:], onesPv, P2[:], start=(pr == 0),
                                 stop=(pr == npr - 1), perf_mode=DR)

            def emit_finish(ic, h2p, rsp, split=False):
                """recip + normalize + out-proj + epilogue + DMA for pass ic.
                split=True pipelines four quarter-windows (tail latency),
                alternating the normalize between DVE and Pool."""
                HW = IW // 2 if split else IW
                for hw in range(IW // HW):
                    fl = slice(hw * HW, (hw + 1) * HW)
                    sl = slice(ic * IW + hw * HW, ic * IW + (hw + 1) * HW)
                    recipB = sp_.tile([C, HW], f32, tag="recipB",
                                      name=f"rcp{ic}_{hw}", bufs=2)
                    nc.vector.reciprocal_approx_fast(out=recipB[:],
                                                     in_=rsp[:, fl])
                    h2n = sp_.tile([C, HW], f32r, tag="h2n",
                                   name=f"h2n{ic}_{hw}", bufs=2)
                    nc.vector.tensor_tensor(out=h2n[:], in0=h2p[:, fl],
                                            in1=recipB[:], op=OP.mult)
                    pop = ring.tile([C, HW], f32, tag="sT",
                                    name=f"pop{ic}_{hw}")
                    nc.tensor.matmul(pop[:], woutR[:], h2n[:], start=True,
                                     stop=True)
                    outS = sp_.tile([C, HW], f32, tag="outS",
                                    name=f"outS{ic}_{hw}", bufs=2)
                    if split:
                        ob = sp_.tile([C, HW], f32, tag="ob",
                                      name=f"ob{ic}_{hw}", bufs=2)
                        nc.scalar.activation(out=ob[:], in_=pop[:],
                                             func=AF.Identity, bias=beffT[:],
                                             scale=1.0)
                        nc.vector.tensor_tensor(out=outS[:], in0=ob[:],
                                                in1=xS[:, sl], op=OP.add)
                    else:
                        nc.vector.scalar_tensor_tensor(
                            out=outS[:], in0=pop[:], scalar=beffT[:],
                            in1=xS[:, sl], op0=OP.add, op1=OP.add)
                    nc.sync.dma_start(o_d[:, sl], outS[:])

            # ---- attention: flat loop over 4 passes x 32 j-tiles. The PV
            # lag and the per-pass finish both cross pass boundaries in
            # emission order, so PE's QK stream never drains at a boundary.
            vp = None
            vpbase = 0
            h2rs = {}
            pending_finish = None   # finish(ic) is emitted early in pass ic+1
            pend = []               # (ic, pr, P2) awaiting PV/ones, 2-late

            def get_h2rs(ic):
                if ic not in h2rs:
                    h2rs[ic] = (
                        ph2.tile([C, IW], f32, tag="h2", name=f"h2_{ic}"),
                        prs.tile([C, IW], f32, tag="rs", name=f"rs_{ic}"))
                return h2rs[ic]

            def flush_one():
                fic, pr, Pp = pend.pop(0)
                h2p, rsp = get_h2rs(fic)
                emit_pv_ones(h2p, rsp, pr, Pp, NJT // 2)

            for ic in range(NPASS):
                for pr in range(NJT // 2):
                    sT2 = ring.tile([C, 2, IW], f32, tag="sT",
                                    name=f"s{ic}_{pr}")
                    P2 = pP.tile([C, 2, IW], f8, tag="P",
                                 name=f"P{ic}_{pr}")
                    e0 = ASSIGN[NJT * ic + 2 * pr]
                    e1 = ASSIGN[NJT * ic + 2 * pr + 1]
                    for tp in range(2):
                        jt = 2 * pr + tp
                        with tc.high_priority(offset=24):
                            nc.tensor.matmul(
                                sT2[:, tp, :],
                                z2x[:, :, 128 * jt:128 * (jt + 1)],
                                qtM8[:, :, ic * IW:(ic + 1) * IW],
                                start=True, stop=True, perf_mode=DR)
                        if ic == 0:
                            # vT rides the same z-tile ldweights in pass 0
                            if jt % 8 == 0:
                                vp = ring.tile([C, 2 * IW], f32, tag="sT",
                                               name=f"vp{jt // 8}")
                                vpbase = jt // 8
                            nc.tensor.matmul(vp[:, 128 * (jt % 8):
                                                128 * (jt % 8 + 1)],
                                             z2x[:, :,
                                                 128 * jt:128 * (jt + 1)],
                                             wv2x8[:], start=True, stop=True,
                                             perf_mode=DR)
                            if jt % 8 == 7:
                                vce = VTE[vpbase % len(VTE)]
                                if vce == "A":
                                    nc.scalar.activation(
                                        out=vTR[:, 2 * IW * vpbase:
                                                2 * IW * (vpbase + 1)],
                                        in_=vp[:], func=AF.Identity,
                                        scale=1.0)
                                else:
                                    nc.vector.tensor_copy(
                                        vTR[:, 2 * IW * vpbase:
                                            2 * IW * (vpbase + 1)], vp[:])
                        if tp == 0 and e0 != e1:
                            emit_exp(sT2, P2, slice(0, 1), e0)
                    if e0 == e1:
                        emit_exp(sT2, P2, slice(0, 2), e0)  # fused pair op
                    else:
                        emit_exp(sT2, P2, slice(1, 2), e1)
                    pend.append((ic, pr, P2))
                    # PV/ones lag: pass ic's last pairs run into pass ic+1,
                    # giving the recip/normalize drain of h2/rs (bufs=1)
                    # half a pass of slack
                    while len(pend) > PVLAG:
                        flush_one()
                    # previous pass's finish rides mid-pass so its out-proj
                    # matmul (gated by the recip/normalize chain) never
                    # blocks this pass's QK stream
                    if pr == 5 and pending_finish is not None:
                        emit_finish(*pending_finish)
                        pending_finish = None
                    # window ic+1's qtM rides mid-pass ic (needed only
                    # at pass ic+1), spreading conversion work across passes
                    if pr == 7 and ic < NPASS - 1:
                        emit_qtm(ic + 1, QTE[ic + 1])
                pending_finish = (ic,) + get_h2rs(ic)
            while pend:
                flush_one()
            emit_finish(*pending_finish, split=True)

    nc.compile()
    return nc


def host_inputs(x, gn_w, gn_b, w_qkv, b_qkv, w_out, b_out):
    """Build the 8 per-core input maps from the full problem inputs."""
    x = np.asarray(x, dtype=np.float32)
    B, _, N = x.shape
    S = N // 2
    w_qkv = np.asarray(w_qkv, np.float32)
    w_out = np.asarray(w_out, np.float32)
    b_qkv = np.asarray(b_qkv, np.float32)
    b_out = np.asarray(b_out, np.float32)
    gn_w = np.asarray(gn_w, np.float32)
    gn_b = np.asarray(gn_b, np.float32)

    M = (w_qkv[0:C].T @ w_qkv[C:2 * C]).astype(np.float32)   # [C, C]
    bqt = (w_qkv[C:2 * C].T @ b_qkv[0:C]).astype(np.float32)  # [C] k-side
    wv = w_qkv[2 * C:3 * C]                                   # [C, C]
    bv = b_qkv[2 * C:3 * C]
    perm = np.concatenate([np.arange(0, C, 2), np.arange(1, C, 2)])  # 2p+t
    gidx = np.arange(C) // GS
    gmask = (gidx[:, None] == gidx[None, :]).astype(np.float32) / GS

    b_eff = b_out + w_out @ bv
    bcat = np.stack([np.zeros(C, np.float32), np.zeros(C, np.float32),
                     b_eff, gn_w, gn_b], axis=1).astype(np.float32)
    # wcat: [Mc | wv^T | wout^T | gmask2x | gmask | bcat]
    wcat = np.concatenate(
        [M[:, perm], wv.T, w_out.T, gmask[:, perm], gmask, bcat],
        axis=1).astype(np.float32)
    # wcat2: [M2xc | wv2x | bcat2]; M2xc[p, t*C+o] = M[2p+t, perm(o)],
    # wv2x[p, t*C+cv] = wv[cv, 2p+t]
    M2xc = M[:, perm].reshape(NP, 2 * C)
    wv2x = wv.T.reshape(NP, 2 * C)
    bcat2 = np.concatenate([gn_w.reshape(NP, 2), bqt.reshape(NP, 2)],
                           axis=1).astype(np.float32)
    wcat2 = np.concatenate([M2xc, wv2x, bcat2], axis=1).astype(np.float32)

    in_maps = []
    for core in range(N_CORES):
        b, half = divmod(core, 2)
        xb = np.roll(x[b], -half * S, axis=1)
        in_maps.append({"x": np.ascontiguousarray(xb), "wcat": wcat,
                        "wcat2": wcat2})
    return in_maps


_NC_CACHE = {}
_RUNNER_CACHE = {}


def _make_runner(nc):
    """Compile-once runner: replicates bass2jax.run_bass_via_pjrt but keeps the
    jitted sharded callable so repeat executions skip recompilation."""
    import jax
    import concourse.mybir as mybir
    from jax.sharding import Mesh, PartitionSpec
    from jax.experimental.shard_map import shard_map
    from concourse.bass2jax import (_bass_exec_p, install_neuronx_cc_hook,
                                    partition_id_tensor)

    install_neuronx_cc_hook()
    partition_name = nc.partition_id_tensor.name if nc.partition_id_tensor else None
    in_names, out_names, out_avals, zero_shapes = [], [], [], []
    for alloc in nc.m.functions[0].allocations:
        if not isinstance(alloc, mybir.MemoryLocationSet):
            continue
        name = alloc.memorylocations[0].name
        if alloc.kind == "ExternalInput":
            if name == partition_name:
                continue
            in_names.append(name)
        elif alloc.kind == "ExternalOutput":
            out_names.append(name)
            shape = tuple(alloc.tensor_shape)
            dtype = mybir.dt.np(alloc.dtype)
            out_avals.append(jax.core.ShapedArray(shape, dtype))
            zero_shapes.append((shape, dtype))
    n_params = len(in_names)
    all_names = in_names + out_names
    if partition_name is not None:
        all_names = all_names + [partition_name]
    donate = tuple(range(n_params, n_params + len(out_names)))

    def _body(*args):
        operands = list(args)
        if partition_name is not None:
            operands.append(partition_id_tensor())
        return tuple(_bass_exec_p.bind(
            *operands, out_avals=tuple(out_avals), in_names=tuple(all_names),
            out_names=tuple(out_names), lowering_input_output_aliases=(),
            sim_require_finite=True, sim_require_nnan=True, nc=nc))

    devices = jax.devices()[:N_CORES]
    mesh = Mesh(np.asarray(devices), ("core",))
    specs = (PartitionSpec("core"),)
    sharded = jax.jit(
        shard_map(_body, mesh=mesh,
                  in_specs=specs * (n_params + len(out_names)),
                  out_specs=specs * len(out_names), check_rep=False),
        donate_argnums=donate, keep_unused=True)

    def run(in_maps):
        concat_in = [np.concatenate([np.asarray(m[nm]) for m in in_maps], axis=0)
                     for nm in in_names]
        concat_zeros = [np.zeros((N_CORES * s[0], *s[1:]), d) for s, d in zero_shapes]
        out_arrs = sharded(*concat_in, *concat_zeros)
        out_arrs = [np.asarray(a) for a in out_arrs]
        return [{nm: out_arrs[i].reshape(N_CORES, *out_avals[i].shape)[c]
                 for i, nm in enumerate(out_names)} for c in range(N_CORES)]

    return run


def get_runner(N=4096):
    if N not in _RUNNER_CACHE:
        if N not in _NC_CACHE:
            _NC_CACHE[N] = build(N)
        _RUNNER_CACHE[N] = _make_runner(_NC_CACHE[N])
    return _RUNNER_CACHE[N]


def kernel(x, gn_w, gn_b, w_qkv, b_qkv, w_out, b_out):
    from concourse._compat import axon_active

    x = np.asarray(x, dtype=np.float32)
    B, _, N = x.shape
    S = N // 2
    in_maps = host_inputs(x, gn_w, gn_b, w_qkv, b_qkv, w_out, b_out)
    if axon_active():
        results = get_runner(N)(in_maps)
    else:
        from concourse.bass_utils import run_bass_kernel_spmd

        if N not in _NC_CACHE:
            _NC_CACHE[N] = build(N)
        results = run_bass_kernel_spmd(_NC_CACHE[N], in_maps,
                                       core_ids=list(range(N_CORES))).results
    out = np.empty((B, C, N), dtype=np.float32)
    for core in range(N_CORES):
        b, half = divmod(core, 2)
        out[b, :, half * S:(half + 1) * S] = results[core]["out"]
    return out


# revision 6
# speedup vs baseline: 1.1640x; 1.0118x over previous
"""AttentionBlock (GroupNorm -> QKV -> full attention -> out-proj + residual)
for B=4, C=128, N=4096 on 8 Trainium2 NeuronCores.

Sharding: 8 cores = 4 batches x 2 query-slabs of N/2. Every core runs the
same program; the host rolls each core's x so its query slab is always
columns [0, N/2).

Key moves:
- All big matmuls are fp8 DoubleRow (0.5 cyc/col): channels split as
  c = 2p + t into a [64, 2, *] layout so the C=128 contraction rides the
  256-row DoubleRow path. z = a*x (GN scale only; the shift b folds into a
  per-channel bias u = M^T b + wk^T bq added during the qtM psum->fp8
  conversion, since scores s[j,i] = z_j . (M^T z_i + u)).
- exp is split across the two PSUM-capable engines: ACT runs real Exp,
  DVE runs a Schraudolph exp (int8(A*s + B) bitcast as fp8e4m3, one
  tensor_scalar op). GPSIMD cannot touch PSUM (and has no TensorScalar),
  so it only carries SBUF-side const conversions. Scores for a j-tile
  pair land in one [C, 2, 512] PSUM tile; a same-engine pair is one
  fused 1024-wide exp op. P pairs feed fp8 DoubleRow PV and rowsum
  (ones) matmuls directly - no DVE pair-adds anywhere.
- One PSUM layout for the whole kernel: a 3-slot ring of 2-bank pair
  tiles (scores, qtM/vT staging, out-proj) + 1 bank each for the PV and
  rowsum accumulators. No mid-kernel pool transitions.
- v comes from z via wv fp8 DoubleRow sharing QK's ldweights in pass 0;
  GroupNorm stats run once in 128-layout (exact b) with a dup'd Newton
  chain in the split layout for a; psum->fp8 conversions are spread
  A/D by a tuned schedule; qtM windows 2..4 ride earlier passes.
- Per-pass (i-window 512) finish: recip + normalize + epilogue (DVE),
  out-proj (PE), DMA out - emitted mid-next-pass so PE's QK stream never
  stalls at a pass boundary; PV/ones run 6 pairs late for the same
  reason. The last pass's finish is split in halves with an ACT assist.
End-to-end relative error vs the fp32 reference is ~6.4e-4
(fp8-dominated); TimelineSim per-core time ~73.6us (baseline was 94.0us).
"""

import math
import sys

if "/opt/trn_rl_repo" not in sys.path:
    sys.path.insert(0, "/opt/trn_rl_repo")

import numpy as np

C = 128
G = 8
GS = C // G  # channels per group
EPS = 1e-5
N_CORES = 8
NP = C // 2  # 64: partition count of the split-channel layout


def build(N=4096, repeat=1, cfg=None):
    """Build the per-core Bass program. Returns the compiled Bacc module."""
    cfg = dict(cfg or {})
    P0 = cfg.get("p0", (10, 6))           # pass-0 exp PAIR counts A/D
    PS = cfg.get("ps", (10, 6))           # steady-pass exp PAIR counts
    VTE = cfg.get("vte", "AADA")     # vT conv engines by chunk
    QTE = cfg.get("qte", "AADD")         # qtM conv engines by window
    PVLAG = cfg.get("pvlag", 6)          # pairs of PV/ones lag
    import concourse.bacc as bacc
    import concourse.bass as bass
    import concourse.mybir as mybir
    import concourse.tile as tile

    f32 = mybir.dt.float32
    f32r = mybir.dt.float32r
    f8 = mybir.dt.float8e4
    i8 = mybir.dt.int8
    i32 = mybir.dt.int32
    AF = mybir.ActivationFunctionType
    OP = mybir.AluOpType
    DR = mybir.MatmulPerfMode.DoubleRow

    S = N // 2            # query slab width per core
    IW = 512              # i-window per pass
    NPASS = S // IW       # 4
    NJT = N // 128        # 32 j tiles
    XC = 512              # xS DMA/bnstats chunk
    NXC = N // XC         # 8
    X2C = 1024            # x2x DMA / z2x chunk
    NX2 = N // X2C        # 4
    SCALE = 1.0 / math.sqrt(C)
    # Schraudolph: int8 y = trunc(A*s_raw + B) bitcast fp8e4m3 ~ exp(SCALE*s)
    SCH_A = 8.0 * math.log2(math.e) * SCALE
    SCH_B = 8.0 * (7.0 - 0.045) + 0.5

    # exp engine per (pass, jtile): A=ACT, D=DVE, P=Pool, rate-weighted
    # (ACT 0.61us, DVE 0.66, Pool 0.81 per tile) with D/P's extra per-pass
    # work (recip/normalize on D, epilogue on P, conversions in pass 0)
    # subtracted from their shares.
    def mk_assign(na, nd):
        # per-PAIR engine letters, doubled to tiles: same-engine pairs let
        # one amortized pair-op cover both halves
        out, acc = [], {"A": 0.0, "D": 0.0}
        want = {"A": na, "D": nd}
        for i in range(NJT // 2):
            e = max(want, key=lambda k: want[k] * (i + 1) / (NJT // 2) - acc[k])
            acc[e] += 1
            out += [e, e]
        return out

    ASSIGN = mk_assign(*P0)
    for _ in range(NPASS - 1):
        ASSIGN += mk_assign(*PS)

    nc = bacc.Bacc("TRN2", target_bir_lowering=False, debug=False)

    x_d = nc.dram_tensor("x", [C, N], f32, kind="ExternalInput").ap()
    # wcat cols: [Mc | wv^T | wout^T | gmask2x | gmask | bcat(5)]
    w_d = nc.dram_tensor("wcat", [C, 5 * C + 5], f32, kind="ExternalInput").ap()
    # wcat2 cols: [M2xc | wv2x | bcat2(4)]
    w2_d = nc.dram_tensor("wcat2", [NP, 4 * C + 4], f32,
                          kind="ExternalInput").ap()
    o_d = nc.dram_tensor("out", [C, S], f32, kind="ExternalOutput").ap()

    with tile.TileContext(nc) as tc:
        with tc.tile_pool(name="consts", bufs=1) as cp, \
             tc.tile_pool(name="big", bufs=1) as bp, \
             tc.tile_pool(name="small", bufs=3) as sp_, \
             tc.tile_pool(name="pP", bufs=10) as pP, \
             tc.tile_pool(name="ring", bufs=3, space="PSUM") as ring, \
             tc.tile_pool(name="ph2", bufs=1, space="PSUM") as ph2, \
             tc.tile_pool(name="prs", bufs=1, space="PSUM") as prs:
            # ---- DMAs: xS chunks first (stats chain), consts, x2x ----
            xS = bp.tile([C, N], f32, tag="x")
            for c in range(4):
                nc.sync.dma_start(xS[:, c * 2 * XC:(c + 1) * 2 * XC],
                                  x_d[:, c * 2 * XC:(c + 1) * 2 * XC])
            wS = cp.tile([C, 5 * C + 5], f32, tag="w")
            nc.sync.dma_start(wS[:], w_d[:])
            bS = wS[:, 5 * C:5 * C + 5]
            x2x = bp.tile([NP, 2, N], f32, tag="x2x")
            x2v = x_d.rearrange("(p two) n -> p two n", two=2)
            nc.sync.dma_start(x2x[:, :, 0:X2C], x2v[:, :, 0:X2C])
            wc2 = cp.tile([NP, 4 * C + 4], f32, tag="w2")
            nc.sync.dma_start(wc2[:], w2_d[:])
            b2S = wc2[:, 4 * C:4 * C + 4]
            for c in range(1, NX2):
                nc.sync.dma_start(x2x[:, :, c * X2C:(c + 1) * X2C],
                                  x2v[:, :, c * X2C:(c + 1) * X2C])

            # ---- ACT exp-table preload (runs during DMA) ----
            trash = sp_.tile([C, 1], f32, tag="trash")
            nc.vector.memset(trash[:], 0.0)
            nc.scalar.activation(out=trash[:], in_=trash[:], func=AF.Exp,
                                 scale=1.0)

            # ---- const conversions ----
            M2xc8 = cp.tile([NP, 2, C], f8, tag="m2xc8")
            nc.gpsimd.tensor_copy(M2xc8[:], wc2[:, 0:2 * C])
            wv2x8 = cp.tile([NP, 2, C], f8, tag="wv2x8")
            nc.gpsimd.tensor_copy(wv2x8[:], wc2[:, 2 * C:4 * C])
            onesP = cp.tile([C, 2 * C], f8, tag="onesp")
            nc.gpsimd.memset(onesP[:], 1.0)
            onesPv = onesP[:].rearrange("p (two c) -> p two c", two=2)
            woutR = cp.tile([C, C], f32r, tag="woutr")
            nc.gpsimd.tensor_copy(woutR[:], wS[:, 2 * C:3 * C])

            # persistent big tensors
            z2x = bp.tile([NP, 2, N], f8, tag="z2x")
            qtM8 = bp.tile([NP, 2, S], f8, tag="qtm8")
            vTR = bp.tile([C, N], f8, tag="vT")

            def rsqrt_chain(eng, shape, gmean, gex2, tag):
                """Magic-Newton rsqrt(var+eps) on [part, k] APs -> inv tile."""
                part, k = shape
                gm2 = sp_.tile([part, k], f32, tag=tag + "gm2")
                eng.tensor_tensor(out=gm2[:], in0=gmean, in1=gmean, op=OP.mult)
                xv = sp_.tile([part, k], f32, tag=tag + "xv")
                eng.tensor_tensor(out=xv[:], in0=gex2, in1=gm2[:],
                                  op=OP.subtract)
                eng.tensor_scalar(out=xv[:], in0=xv[:], scalar1=1.0,
                                  scalar2=EPS, op0=OP.mult, op1=OP.add)
                yh = sp_.tile([part, k], i32, tag=tag + "yh")
                eng.tensor_scalar(out=yh[:], in0=xv[:].bitcast(i32),
                                  scalar1=1, scalar2=None,
                                  op0=OP.logical_shift_right)
                eng.tensor_scalar(out=yh[:], in0=yh[:], scalar1=-1,
                                  scalar2=0x5F3759DF, op0=OP.mult, op1=OP.add)
                y0 = yh[:].bitcast(f32)
                tN = sp_.tile([part, k], f32, tag=tag + "tN")
                eng.tensor_tensor(out=tN[:], in0=y0, in1=y0, op=OP.mult)
                eng.tensor_tensor(out=tN[:], in0=tN[:], in1=xv[:], op=OP.mult)
                eng.tensor_scalar(out=tN[:], in0=tN[:], scalar1=-0.5,
                                  scalar2=1.5, op0=OP.mult, op1=OP.add)
                inv = sp_.tile([part, k], f32, tag=tag + "inv")
                eng.tensor_tensor(out=inv[:], in0=y0, in1=tN[:], op=OP.mult)
                return inv

            # ---- GroupNorm stats: bnstats (DVE) -> group matmuls (PE) ----
            st6 = sp_.tile([C, NXC, 6], f32, tag="st6")
            for c in range(NXC):
                nc.vector.bn_stats(out=st6[:, c, :],
                                   in_=xS[:, c * XC:(c + 1) * XC])
            mv = sp_.tile([C, 2], f32, tag="mv")
            nc.vector.bn_aggr(out=mv[:], in_=st6[:])
            nc.vector.scalar_tensor_tensor(out=mv[:, 1:2], in0=mv[:, 0:1],
                                           scalar=mv[:, 0:1], in1=mv[:, 1:2],
                                           op0=OP.mult, op1=OP.add)
            gps = ring.tile([C, 2], f32, tag="sT", name="gps")
            nc.tensor.matmul(gps[:], wS[:, 4 * C:5 * C], mv[:], start=True,
                             stop=True)
            gst = sp_.tile([C, 2], f32, tag="gst")
            nc.vector.tensor_copy(gst[:], gps[:])
            gap = ring.tile([NP, 2, 2], f32, tag="sT", name="gap")
            for t in range(2):
                nc.tensor.matmul(gap[:, t, :],
                                 wS[:, 3 * C + NP * t:3 * C + NP * (t + 1)],
                                 mv[:], start=True, stop=True)
            gas = sp_.tile([NP, 2, 2], f32, tag="gas")
            nc.vector.tensor_copy(gas[:], gap[:])

            # split layout scale a2x on DVE; exact 128-layout b on Pool
            inv2x = rsqrt_chain(nc.vector, [NP, 2], gas[:, :, 0:1],
                                gas[:, :, 1:2], "n2")
            a2x = sp_.tile([NP, 2], f32, tag="a2x")
            nc.vector.tensor_tensor(out=a2x[:], in0=b2S[:, 0:2], in1=inv2x[:],
                                    op=OP.mult)
            inv128 = rsqrt_chain(nc.vector, [C, 1], gst[:, 0:1], gst[:, 1:2],
                                 "n1")
            aT = sp_.tile([C, 1], f32, tag="aT")
            nc.vector.tensor_tensor(out=aT[:], in0=bS[:, 3:4], in1=inv128[:],
                                    op=OP.mult)
            bT = sp_.tile([C, 1], f32, tag="bT")
            nc.vector.tensor_tensor(out=bT[:], in0=gst[:, 0:1], in1=aT[:],
                                    op=OP.mult)
            nc.vector.tensor_tensor(out=bT[:], in0=bS[:, 4:5], in1=bT[:],
                                    op=OP.subtract)

            # u2x = (M^T b + bqt) in split layout; beff = bcat + w_out wv^T b
            u_p = ring.tile([NP, 2], f32, tag="sT", name="u_p")
            for t in range(2):
                nc.tensor.matmul(u_p[:, t:t + 1], wS[:, NP * t:NP * (t + 1)],
                                 bT[:], start=True, stop=True)
            u2x = sp_.tile([NP, 2], f32, tag="u2xf")
            nc.vector.tensor_tensor(out=u2x[:], in0=u_p[:], in1=b2S[:, 2:4],
                                    op=OP.add)
            vc_p = ring.tile([C, 2], f32, tag="sT", name="vc_p")
            nc.tensor.matmul(vc_p[:, 0:1], wS[:, C:2 * C], bT[:], start=True,
                             stop=True)
            vcS = sp_.tile([C, 1], f32, tag="vcS")
            nc.vector.tensor_copy(vcS[:], vc_p[:, 0:1])
            wov_p = ring.tile([C, 2], f32, tag="sT", name="wov_p")
            nc.tensor.matmul(wov_p[:, 0:1], wS[:, 2 * C:3 * C], vcS[:],
                             start=True, stop=True)
            beffT = sp_.tile([C, 1], f32, tag="beffT")
            nc.vector.tensor_tensor(out=beffT[:], in0=bS[:, 2:3],
                                    in1=wov_p[:, 0:1], op=OP.add)

            # ---- z2x = a2x * x2x -> fp8 (all on DVE: 2x-port mode there).
            # high_priority so the scheduler never defers z chunks behind
            # pass-0 exps (z gates every QK of the j-tile it covers).
            with tc.high_priority():
                for c in range(NX2):
                    sl = slice(c * X2C, (c + 1) * X2C)
                    for t in range(2):
                        nc.vector.tensor_scalar(out=z2x[:, t, sl],
                                                in0=x2x[:, t, sl],
                                                scalar1=a2x[:, t:t + 1],
                                                scalar2=None, op0=OP.mult)

            def emit_qtm(ic, conv):
                """qtM for i-window ic: 2 DR matmuls + 2 psum->fp8(+u) convs.
                conv in {'D','P','A'}."""
                sl = slice(ic * IW, (ic + 1) * IW)
                for t in range(2):
                    qp = ring.tile([NP, IW], f32, tag="sT",
                                   name=f"qp{ic}_{t}")
                    nc.tensor.matmul(qp[:], M2xc8[:, :, NP * t:NP * (t + 1)],
                                     z2x[:, :, sl], start=True, stop=True,
                                     perf_mode=DR)
                    if conv == "A":
                        nc.scalar.activation(out=qtM8[:, t, sl], in_=qp[:],
                                             func=AF.Identity,
                                             bias=u2x[:, t:t + 1], scale=1.0)
                    else:
                        nc.vector.tensor_scalar(out=qtM8[:, t, sl], in0=qp[:],
                                                scalar1=u2x[:, t:t + 1],
                                                scalar2=None, op0=OP.add)

            # qtM for pass 0 first (gates attention start); rest follow
            emit_qtm(0, QTE[0])

            def emit_exp(sT2, P2, sl2, eng):
                """exp over sT2[:, sl2, :] -> P2[:, sl2, :]; sl2 covers one
                or both pair halves (fused when both on one engine)."""
                if eng == "A":
                    nc.scalar.activation(out=P2[:, sl2, :],
                                         in_=sT2[:, sl2, :],
                                         func=AF.Exp, scale=SCALE)
                else:
                    nc.vector.tensor_scalar(
                        out=P2[:, sl2, :].bitcast(i8), in0=sT2[:, sl2, :],
                        scalar1=SCH_A, scalar2=SCH_B,
                        op0=OP.mult, op1=OP.add)

            def emit_pv_ones(h2p, rsp, pr, P2, npr):
                vpair = vTR[:, 256 * pr:256 * (pr + 1)].rearrange(
                    "p (two c) -> p two c", two=2)
                nc.tensor.matmul(h2p[:], vpair, P2[:], start=(pr == 0),
                                 stop=(pr == npr - 1), perf_mode=DR)
                nc.tensor.matmul(rsp[:], onesPv, P2[:], start=(pr == 0),
                                 stop=(pr == npr - 1), perf_mode=DR)

            def emit_finish(ic, h2p, rsp, split=False):
                """recip + normalize + out-proj + epilogue + DMA for pass ic.
                split=True pipelines four quarter-windows (tail latency),
                alternating the normalize between DVE and Pool."""
                HW = IW // 2 if split else IW
                for hw in range(IW // HW):
                    fl = slice(hw * HW, (hw + 1) * HW)
                    sl = slice(ic * IW + hw * HW, ic * IW + (hw + 1) * HW)
                    recipB = sp_.tile([C, HW], f32, tag="recipB",
                                      name=f"rcp{ic}_{hw}", bufs=2)
                    nc.vector.reciprocal_approx_fast(out=recipB[:],
                                                     in_=rsp[:, fl])
                    h2n = sp_.tile([C, HW], f32r, tag="h2n",
                                   name=f"h2n{ic}_{hw}", bufs=2)
                    nc.vector.tensor_tensor(out=h2n[:], in0=h2p[:, fl],
                                            in1=recipB[:], op=OP.mult)
                    pop = ring.tile([C, HW], f32, tag="sT",
                                    name=f"pop{ic}_{hw}")
                    nc.tensor.matmul(pop[:], woutR[:], h2n[:], start=True,
                                     stop=True)
                    outS = sp_.tile([C, HW], f32, tag="outS",
                                    name=f"outS{ic}_{hw}", bufs=2)
                    if split:
                        ob = sp_.tile([C, HW], f32, tag="ob",
                                      name=f"ob{ic}_{hw}", bufs=2)
                        nc.scalar.activation(out=ob[:], in_=pop[:],
                                             func=AF.Identity, bias=beffT[:],
                                             scale=1.0)
                        nc.vector.tensor_tensor(out=outS[:], in0=ob[:],
                                                in1=xS[:, sl], op=OP.add)
                    else:
                        nc.vector.scalar_tensor_tensor(
                            out=outS[:], in0=pop[:], scalar=beffT[:],
                            in1=xS[:, sl], op0=OP.add, op1=OP.add)
                    nc.sync.dma_start(o_d[:, sl], outS[:])

            # ---- attention: flat loop over 4 passes x 32 j-tiles. The PV
            # lag and the per-pass finish both cross pass boundaries in
            # emission order, so PE's QK stream never drains at a boundary.
            vp = None
            vpbase = 0
            h2rs = {}
            pending_finish = None   # finish(ic) is emitted early in pass ic+1
            pend = []               # (ic, pr, P2) awaiting PV/ones, 2-late

            def get_h2rs(ic):
                if ic not in h2rs:
                    h2rs[ic] = (
                        ph2.tile([C, IW], f32, tag="h2", name=f"h2_{ic}"),
                        prs.tile([C, IW], f32, tag="rs", name=f"rs_{ic}"))
                return h2rs[ic]

            def flush_one():
                fic, pr, Pp = pend.pop(0)
                h2p, rsp = get_h2rs(fic)
                emit_pv_ones(h2p, rsp, pr, Pp, NJT // 2)

            for ic in range(NPASS):
                for pr in range(NJT // 2):
                    sT2 = ring.tile([C, 2, IW], f32, tag="sT",
                                    name=f"s{ic}_{pr}")
                    P2 = pP.tile([C, 2, IW], f8, tag="P",
                                 name=f"P{ic}_{pr}")
                    e0 = ASSIGN[NJT * ic + 2 * pr]
                    e1 = ASSIGN[NJT * ic + 2 * pr + 1]
                    for tp in range(2):
                        jt = 2 * pr + tp
                        with tc.high_priority(offset=24):
                            nc.tensor.matmul(
                                sT2[:, tp, :],
                                z2x[:, :, 128 * jt:128 * (jt + 1)],
                                qtM8[:, :, ic * IW:(ic + 1) * IW],
                                start=True, stop=True, perf_mode=DR)
                        if ic == 0:
                            # vT rides the same z-tile ldweights in pass 0
                            if jt % 8 == 0:
                                vp = ring.tile([C, 2 * IW], f32, tag="sT",
                                               name=f"vp{jt // 8}")
                                vpbase = jt // 8
                            nc.tensor.matmul(vp[:, 128 * (jt % 8):
                                                128 * (jt % 8 + 1)],
                                             z2x[:, :,
                                                 128 * jt:128 * (jt + 1)],
                                             wv2x8[:], start=True, stop=True,
                                             perf_mode=DR)
                            if jt % 8 == 7:
                                vce = VTE[vpbase % len(VTE)]
                                if vce == "A":
                                    nc.scalar.activation(
                                        out=vTR[:, 2 * IW * vpbase:
                                                2 * IW * (vpbase + 1)],
                                        in_=vp[:], func=AF.Identity,
                                        scale=1.0)
                                else:
                                    nc.vector.tensor_copy(
                                        vTR[:, 2 * IW * vpbase:
                                            2 * IW * (vpbase + 1)], vp[:])
                        if tp == 0 and e0 != e1:
                            emit_exp(sT2, P2, slice(0, 1), e0)
                    if e0 == e1:
                        emit_exp(sT2, P2, slice(0, 2), e0)  # fused pair op
                    else:
                        emit_exp(sT2, P2, slice(1, 2), e1)
                    pend.append((ic, pr, P2))
                    # PV/ones lag: pass ic's last pairs run into pass ic+1,
                    # giving the recip/normalize drain of h2/rs (bufs=1)
                    # half a pass of slack
                    while len(pend) > PVLAG:
                        flush_one()
                    # previous pass's finish rides mid-pass so its out-proj
                    # matmul (gated by the recip/normalize chain) never
                    # blocks this pass's QK stream
                    if pr == 5 and pending_finish is not None:
                        emit_finish(*pending_finish)
                        pending_finish = None
                    # window ic+1's qtM rides mid-pass ic (needed only
                    # at pass ic+1), spreading conversion work across passes
                    if pr == 7 and ic < NPASS - 1:
                        emit_qtm(ic + 1, QTE[ic + 1])
                pending_finish = (ic,) + get_h2rs(ic)
            while pend:
                flush_one()
            emit_finish(*pending_finish, split=True)

    nc.compile()
    return nc


def host_inputs(x, gn_w, gn_b, w_qkv, b_qkv, w_out, b_out):
    """Build the 8 per-core input maps from the full problem inputs."""
    x = np.asarray(x, dtype=np.float32)
    B, _, N = x.shape
    S = N // 2
    w_qkv = np.asarray(w_qkv, np.float32)
    w_out = np.asarray(w_out, np.float32)
    b_qkv = np.asarray(b_qkv, np.float32)
    b_out = np.asarray(b_out, np.float32)
    gn_w = np.asarray(gn_w, np.float32)
    gn_b = np.asarray(gn_b, np.float32)

    M = (w_qkv[0:C].T @ w_qkv[C:2 * C]).astype(np.float32)   # [C, C]
    bqt = (w_qkv[C:2 * C].T @ b_qkv[0:C]).astype(np.float32)  # [C] k-side
    wv = w_qkv[2 * C:3 * C]                                   # [C, C]
    bv = b_qkv[2 * C:3 * C]
    perm = np.concatenate([np.arange(0, C, 2), np.arange(1, C, 2)])  # 2p+t
    gidx = np.arange(C) // GS
    gmask = (gidx[:, None] == gidx[None, :]).astype(np.float32) / GS

    b_eff = b_out + w_out @ bv
    bcat = np.stack([np.zeros(C, np.float32), np.zeros(C, np.float32),
                     b_eff, gn_w, gn_b], axis=1).astype(np.float32)
    # wcat: [Mc | wv^T | wout^T | gmask2x | gmask | bcat]
    wcat = np.concatenate(
        [M[:, perm], wv.T, w_out.T, gmask[:, perm], gmask, bcat],
        axis=1).astype(np.float32)
    # wcat2: [M2xc | wv2x | bcat2]; M2xc[p, t*C+o] = M[2p+t, perm(o)],
    # wv2x[p, t*C+cv] = wv[cv, 2p+t]
    M2xc = M[:, perm].reshape(NP, 2 * C)
    wv2x = wv.T.reshape(NP, 2 * C)
    bcat2 = np.concatenate([gn_w.reshape(NP, 2), bqt.reshape(NP, 2)],
                           axis=1).astype(np.float32)
    wcat2 = np.concatenate([M2xc, wv2x, bcat2], axis=1).astype(np.float32)

    in_maps = []
    for core in range(N_CORES):
        b, half = divmod(core, 2)
        xb = np.roll(x[b], -half * S, axis=1)
        in_maps.append({"x": np.ascontiguousarray(xb), "wcat": wcat,
                        "wcat2": wcat2})
    return in_maps


_NC_CACHE = {}
_RUNNER_CACHE = {}


def _make_runner(nc):
    """Compile-once runner: replicates bass2jax.run_bass_via_pjrt but keeps the
    jitted sharded callable so repeat executions skip recompilation."""
    import jax
    import concourse.mybir as mybir
    from jax.sharding import Mesh, PartitionSpec
    from jax.experimental.shard_map import shard_map
    from concourse.bass2jax import (_bass_exec_p, install_neuronx_cc_hook,
                                    partition_id_tensor)

    install_neuronx_cc_hook()
    partition_name = nc.partition_id_tensor.name if nc.partition_id_tensor else None
    in_names, out_names, out_avals, zero_shapes = [], [], [], []
    for alloc in nc.m.functions[0].allocations:
        if not isinstance(alloc, mybir.MemoryLocationSet):
            continue
        name = alloc.memorylocations[0].name
        if alloc.kind == "ExternalInput":
            if name == partition_name:
                continue
            in_names.append(name)
        elif alloc.kind == "ExternalOutput":
            out_names.append(name)
            shape = tuple(alloc.tensor_shape)
            dtype = mybir.dt.np(alloc.dtype)
            out_avals.append(jax.core.ShapedArray(shape, dtype))
            zero_shapes.append((shape, dtype))
    n_params = len(in_names)
    all_names = in_names + out_names
    if partition_name is not None:
        all_names = all_names + [partition_name]
    donate = tuple(range(n_params, n_params + len(out_names)))

    def _body(*args):
        operands = list(args)
        if partition_name is not None:
            operands.append(partition_id_tensor())
        return tuple(_bass_exec_p.bind(
            *operands, out_avals=tuple(out_avals), in_names=tuple(all_names),
            out_names=tuple(out_names), lowering_input_output_aliases=(),
            sim_require_finite=True, sim_require_nnan=True, nc=nc))

    devices = jax.devices()[:N_CORES]
    mesh = Mesh(np.asarray(devices), ("core",))
    specs = (PartitionSpec("core"),)
    sharded = jax.jit(
        shard_map(_body, mesh=mesh,
                  in_specs=specs * (n_params + len(out_names)),
                  out_specs=specs * len(out_names), check_rep=False),
        donate_argnums=donate, keep_unused=True)

    def run(in_maps):
        concat_in = [np.concatenate([np.asarray(m[nm]) for m in in_maps], axis=0)
                     for nm in in_names]
        concat_zeros = [np.zeros((N_CORES * s[0], *s[1:]), d) for s, d in zero_shapes]
        out_arrs = sharded(*concat_in, *concat_zeros)
        out_arrs = [np.asarray(a) for a in out_arrs]
        return [{nm: out_arrs[i].reshape(N_CORES, *out_avals[i].shape)[c]
                 for i, nm in enumerate(out_names)} for c in range(N_CORES)]

    return run


def get_runner(N=4096):
    if N not in _RUNNER_CACHE:
        if N not in _NC_CACHE:
            _NC_CACHE[N] = build(N)
        _RUNNER_CACHE[N] = _make_runner(_NC_CACHE[N])
    return _RUNNER_CACHE[N]


def kernel(x, gn_w, gn_b, w_qkv, b_qkv, w_out, b_out):
    from concourse._compat import axon_active

    x = np.asarray(x, dtype=np.float32)
    B, _, N = x.shape
    S = N // 2
    in_maps = host_inputs(x, gn_w, gn_b, w_qkv, b_qkv, w_out, b_out)
    if axon_active():
        results = get_runner(N)(in_maps)
    else:
        from concourse.bass_utils import run_bass_kernel_spmd

        if N not in _NC_CACHE:
            _NC_CACHE[N] = build(N)
        results = run_bass_kernel_spmd(_NC_CACHE[N], in_maps,
                                       core_ids=list(range(N_CORES))).results
    out = np.empty((B, C, N), dtype=np.float32)
    for core in range(N_CORES):
        b, half = divmod(core, 2)
        out[b, :, half * S:(half + 1) * S] = results[core]["out"]
    return out


# revision 8
# speedup vs baseline: 1.1762x; 1.0104x over previous
"""AttentionBlock (GroupNorm -> QKV -> full attention -> out-proj + residual)
for B=4, C=128, N=4096 on 8 Trainium2 NeuronCores.

Sharding: 8 cores = 4 batches x 2 query-slabs of N/2. Every core runs the
same program; the host rolls each core's x so its query slab is always
columns [0, N/2).

Key moves:
- All big matmuls are fp8 DoubleRow (0.5 cyc/col): channels split as
  c = 2p + t into a [64, 2, *] layout so the C=128 contraction rides the
  256-row DoubleRow path. z = a*x (GN scale only; the shift b folds into a
  per-channel bias u = M^T b + wk^T bq added during the qtM psum->fp8
  conversion, since scores s[j,i] = z_j . (M^T z_i + u)).
- exp is split across the two PSUM-capable engines: ACT runs real Exp,
  DVE runs a Schraudolph exp (int8(A*s + B) bitcast as fp8e4m3, one
  tensor_scalar op). GPSIMD cannot touch PSUM (and has no TensorScalar),
  so it only carries SBUF-side const conversions. Scores for a j-tile
  pair land in one [C, 2, 512] PSUM tile; a same-engine pair is one
  fused 1024-wide exp op. P pairs feed fp8 DoubleRow PV and rowsum
  (ones) matmuls directly - no DVE pair-adds anywhere.
- One PSUM layout for the whole kernel: a 3-slot ring of 2-bank pair
  tiles (scores, qtM/vT staging, out-proj) + 1 bank each for the PV and
  rowsum accumulators. No mid-kernel pool transitions.
- v comes from z via wv fp8 DoubleRow sharing QK's ldweights in pass 0;
  GroupNorm stats run once in 128-layout (exact b) with a dup'd Newton
  chain in the split layout for a; psum->fp8 conversions are spread
  A/D by a tuned schedule; qtM windows 2..4 ride earlier passes.
- Per-pass (i-window 512) finish: recip + normalize + epilogue (DVE),
  out-proj (PE), DMA out - emitted mid-next-pass so PE's QK stream never
  stalls at a pass boundary; PV/ones run 6 pairs late for the same
  reason. The last pass's finish is split in halves with an ACT assist.
End-to-end relative error vs the fp32 reference is ~6.4e-4
(fp8-dominated); TimelineSim per-core time ~73.6us (baseline was 94.0us).
"""

import math
import sys

if "/opt/trn_rl_repo" not in sys.path:
    sys.path.insert(0, "/opt/trn_rl_repo")

import numpy as np

C = 128
G = 8
GS = C // G  # channels per group
EPS = 1e-5
N_CORES = 8
NP = C // 2  # 64: partition count of the split-channel layout


def build(N=4096, repeat=1, cfg=None):
    """Build the per-core Bass program. Returns the compiled Bacc module."""
    cfg = dict(cfg or {})
    P0 = cfg.get("p0", (9, 7))           # pass-0 exp PAIR counts A/D
    PS = cfg.get("ps", (10, 6))           # steady-pass exp PAIR counts
    VTE = cfg.get("vte", "AADA")     # vT conv engines by chunk
    QTE = cfg.get("qte", "AADD")         # qtM conv engines by window
    PVLAG = cfg.get("pvlag", 6)          # pairs of PV/ones lag
    import concourse.bacc as bacc
    import concourse.bass as bass
    import concourse.mybir as mybir
    import concourse.tile as tile

    f32 = mybir.dt.float32
    f32r = mybir.dt.float32r
    f8 = mybir.dt.float8e4
    i8 = mybir.dt.int8
    i32 = mybir.dt.int32
    AF = mybir.ActivationFunctionType
    OP = mybir.AluOpType
    DR = mybir.MatmulPerfMode.DoubleRow

    S = N // 2            # query slab width per core
    IW = 512              # i-window per pass
    NPASS = S // IW       # 4
    NJT = N // 128        # 32 j tiles
    XC = 512              # xS DMA/bnstats chunk
    NXC = N // XC         # 8
    X2C = 1024            # x2x DMA / z2x chunk
    NX2 = N // X2C        # 4
    SCALE = 1.0 / math.sqrt(C)
    # Schraudolph: int8 y = trunc(A*s_raw + B) bitcast fp8e4m3 ~ exp(SCALE*s)
    SCH_A = 8.0 * math.log2(math.e) * SCALE
    SCH_B = 8.0 * (7.0 - 0.045) + 0.5

    # exp engine per (pass, jtile): A=ACT, D=DVE, P=Pool, rate-weighted
    # (ACT 0.61us, DVE 0.66, Pool 0.81 per tile) with D/P's extra per-pass
    # work (recip/normalize on D, epilogue on P, conversions in pass 0)
    # subtracted from their shares.
    def mk_assign(na, nd):
        # per-PAIR engine letters, doubled to tiles: same-engine pairs let
        # one amortized pair-op cover both halves
        out, acc = [], {"A": 0.0, "D": 0.0}
        want = {"A": na, "D": nd}
        for i in range(NJT // 2):
            e = max(want, key=lambda k: want[k] * (i + 1) / (NJT // 2) - acc[k])
            acc[e] += 1
            out += [e, e]
        return out

    ASSIGN = mk_assign(*P0)
    for _ in range(NPASS - 1):
        ASSIGN += mk_assign(*PS)

    nc = bacc.Bacc("TRN2", target_bir_lowering=False, debug=False)

    x_d = nc.dram_tensor("x", [C, N], f32, kind="ExternalInput").ap()
    # wcat cols: [Mc | wv^T | wout^T | gmask2x | gmask | bcat(5)]
    w_d = nc.dram_tensor("wcat", [C, 5 * C + 5], f32, kind="ExternalInput").ap()
    # wcat2 cols: [M2xc | wv2x | bcat2(4)]
    w2_d = nc.dram_tensor("wcat2", [NP, 4 * C + 4], f32,
                          kind="ExternalInput").ap()
    o_d = nc.dram_tensor("out", [C, S], f32, kind="ExternalOutput").ap()

    with tile.TileContext(nc) as tc:
        with tc.tile_pool(name="consts", bufs=1) as cp, \
             tc.tile_pool(name="big", bufs=1) as bp, \
             tc.tile_pool(name="small", bufs=3) as sp_, \
             tc.tile_pool(name="pP", bufs=10) as pP, \
             tc.tile_pool(name="ring", bufs=3, space="PSUM") as ring, \
             tc.tile_pool(name="ph2", bufs=1, space="PSUM") as ph2, \
             tc.tile_pool(name="prs", bufs=1, space="PSUM") as prs:
            # ---- DMAs: xS chunks first (stats chain), consts, x2x ----
            xS = bp.tile([C, N], f32, tag="x")
            for c in range(NXC):
                nc.sync.dma_start(xS[:, c * XC:(c + 1) * XC],
                                  x_d[:, c * XC:(c + 1) * XC])
            wS = cp.tile([C, 5 * C + 5], f32, tag="w")
            nc.sync.dma_start(wS[:], w_d[:])
            bS = wS[:, 5 * C:5 * C + 5]
            x2x = bp.tile([NP, 2, N], f32, tag="x2x")
            x2v = x_d.rearrange("(p two) n -> p two n", two=2)
            nc.sync.dma_start(x2x[:, :, 0:X2C], x2v[:, :, 0:X2C])
            wc2 = cp.tile([NP, 4 * C + 4], f32, tag="w2")
            nc.sync.dma_start(wc2[:], w2_d[:])
            b2S = wc2[:, 4 * C:4 * C + 4]
            for c in range(1, NX2):
                nc.sync.dma_start(x2x[:, :, c * X2C:(c + 1) * X2C],
                                  x2v[:, :, c * X2C:(c + 1) * X2C])

            # ---- ACT exp-table preload (runs during DMA) ----
            trash = sp_.tile([C, 1], f32, tag="trash")
            nc.vector.memset(trash[:], 0.0)
            nc.scalar.activation(out=trash[:], in_=trash[:], func=AF.Exp,
                                 scale=1.0)

            # ---- const conversions ----
            M2xc8 = cp.tile([NP, 2, C], f8, tag="m2xc8")
            nc.gpsimd.tensor_copy(M2xc8[:], wc2[:, 0:2 * C])
            wv2x8 = cp.tile([NP, 2, C], f8, tag="wv2x8")
            nc.gpsimd.tensor_copy(wv2x8[:], wc2[:, 2 * C:4 * C])
            onesP = cp.tile([C, 2 * C], f8, tag="onesp")
            nc.gpsimd.memset(onesP[:], 1.0)
            onesPv = onesP[:].rearrange("p (two c) -> p two c", two=2)
            woutR = cp.tile([C, C], f32r, tag="woutr")
            nc.gpsimd.tensor_copy(woutR[:], wS[:, 2 * C:3 * C])

            # persistent big tensors
            z2x = bp.tile([NP, 2, N], f8, tag="z2x")
            qtM8 = bp.tile([NP, 2, S], f8, tag="qtm8")
            vTR = bp.tile([C, N], f8, tag="vT")

            def rsqrt_chain(eng, shape, gmean, gex2, tag):
                """Magic-Newton rsqrt(var+eps) on [part, k] APs -> inv tile."""
                part, k = shape
                gm2 = sp_.tile([part, k], f32, tag=tag + "gm2")
                eng.tensor_tensor(out=gm2[:], in0=gmean, in1=gmean, op=OP.mult)
                xv = sp_.tile([part, k], f32, tag=tag + "xv")
                eng.tensor_tensor(out=xv[:], in0=gex2, in1=gm2[:],
                                  op=OP.subtract)
                eng.tensor_scalar(out=xv[:], in0=xv[:], scalar1=1.0,
                                  scalar2=EPS, op0=OP.mult, op1=OP.add)
                yh = sp_.tile([part, k], i32, tag=tag + "yh")
                eng.tensor_scalar(out=yh[:], in0=xv[:].bitcast(i32),
                                  scalar1=1, scalar2=None,
                                  op0=OP.logical_shift_right)
                eng.tensor_scalar(out=yh[:], in0=yh[:], scalar1=-1,
                                  scalar2=0x5F3759DF, op0=OP.mult, op1=OP.add)
                y0 = yh[:].bitcast(f32)
                tN = sp_.tile([part, k], f32, tag=tag + "tN")
                eng.tensor_tensor(out=tN[:], in0=y0, in1=y0, op=OP.mult)
                eng.tensor_tensor(out=tN[:], in0=tN[:], in1=xv[:], op=OP.mult)
                eng.tensor_scalar(out=tN[:], in0=tN[:], scalar1=-0.5,
                                  scalar2=1.5, op0=OP.mult, op1=OP.add)
                inv = sp_.tile([part, k], f32, tag=tag + "inv")
                eng.tensor_tensor(out=inv[:], in0=y0, in1=tN[:], op=OP.mult)
                return inv

            # ---- GroupNorm stats: bnstats (DVE) -> group matmuls (PE) ----
            st6 = sp_.tile([C, NXC, 6], f32, tag="st6")
            for c in range(NXC):
                nc.vector.bn_stats(out=st6[:, c, :],
                                   in_=xS[:, c * XC:(c + 1) * XC])
            mv = sp_.tile([C, 2], f32, tag="mv")
            nc.vector.bn_aggr(out=mv[:], in_=st6[:])
            nc.vector.scalar_tensor_tensor(out=mv[:, 1:2], in0=mv[:, 0:1],
                                           scalar=mv[:, 0:1], in1=mv[:, 1:2],
                                           op0=OP.mult, op1=OP.add)
            gps = ring.tile([C, 2], f32, tag="sT", name="gps")
            nc.tensor.matmul(gps[:], wS[:, 4 * C:5 * C], mv[:], start=True,
                             stop=True)
            gap = ring.tile([NP, 2, 2], f32, tag="sT", name="gap")
            for t in range(2):
                nc.tensor.matmul(gap[:, t, :],
                                 wS[:, 3 * C + NP * t:3 * C + NP * (t + 1)],
                                 mv[:], start=True, stop=True)

            # split layout scale a2x first (it gates z2x); single copies
            # out of PSUM (ALU ops may read at most one PSUM operand)
            gas = sp_.tile([NP, 2, 2], f32, tag="gas")
            nc.vector.tensor_copy(gas[:], gap[:])
            inv2x = rsqrt_chain(nc.vector, [NP, 2], gas[:, :, 0:1],
                                gas[:, :, 1:2], "n2")
            a2x = sp_.tile([NP, 2], f32, tag="a2x")
            nc.vector.tensor_tensor(out=a2x[:], in0=b2S[:, 0:2], in1=inv2x[:],
                                    op=OP.mult)
            gst = sp_.tile([C, 2], f32, tag="gst")
            nc.vector.tensor_copy(gst[:], gps[:])
            inv128 = rsqrt_chain(nc.vector, [C, 1], gst[:, 0:1], gst[:, 1:2],
                                 "n1")
            aT = sp_.tile([C, 1], f32, tag="aT")
            nc.vector.tensor_tensor(out=aT[:], in0=bS[:, 3:4], in1=inv128[:],
                                    op=OP.mult)
            bT = sp_.tile([C, 1], f32, tag="bT")
            nc.vector.tensor_tensor(out=bT[:], in0=gst[:, 0:1], in1=aT[:],
                                    op=OP.mult)
            nc.vector.tensor_tensor(out=bT[:], in0=bS[:, 4:5], in1=bT[:],
                                    op=OP.subtract)

            # u2x = (M^T b + bqt) in split layout; beff = bcat + w_out wv^T b
            u_p = ring.tile([NP, 2], f32, tag="sT", name="u_p")
            for t in range(2):
                nc.tensor.matmul(u_p[:, t:t + 1], wS[:, NP * t:NP * (t + 1)],
                                 bT[:], start=True, stop=True)
            u2x = sp_.tile([NP, 2], f32, tag="u2xf")
            nc.vector.tensor_tensor(out=u2x[:], in0=u_p[:], in1=b2S[:, 2:4],
                                    op=OP.add)
            vc_p = ring.tile([C, 2], f32, tag="sT", name="vc_p")
            nc.tensor.matmul(vc_p[:, 0:1], wS[:, C:2 * C], bT[:], start=True,
                             stop=True)
            vcS = sp_.tile([C, 1], f32, tag="vcS")
            nc.vector.tensor_copy(vcS[:], vc_p[:, 0:1])
            wov_p = ring.tile([C, 2], f32, tag="sT", name="wov_p")
            nc.tensor.matmul(wov_p[:, 0:1], wS[:, 2 * C:3 * C], vcS[:],
                             start=True, stop=True)
            beffT = sp_.tile([C, 1], f32, tag="beffT")
            nc.vector.tensor_tensor(out=beffT[:], in0=bS[:, 2:3],
                                    in1=wov_p[:, 0:1], op=OP.add)

            # ---- z2x = a2x * x2x -> fp8 (all on DVE: 2x-port mode there).
            # high_priority so the scheduler never defers z chunks behind
            # pass-0 exps (z gates every QK of the j-tile it covers).
            with tc.high_priority():
                for c in range(NX2):
                    sl = slice(c * X2C, (c + 1) * X2C)
                    for t in range(2):
                        nc.vector.tensor_scalar(out=z2x[:, t, sl],
                                                in0=x2x[:, t, sl],
                                                scalar1=a2x[:, t:t + 1],
                                                scalar2=None, op0=OP.mult)

            def emit_qtm(ic, conv):
                """qtM for i-window ic: 2 DR matmuls + 2 psum->fp8(+u) convs.
                conv in {'D','P','A'}."""
                sl = slice(ic * IW, (ic + 1) * IW)
                for t in range(2):
                    qp = ring.tile([NP, IW], f32, tag="sT",
                                   name=f"qp{ic}_{t}")
                    nc.tensor.matmul(qp[:], M2xc8[:, :, NP * t:NP * (t + 1)],
                                     z2x[:, :, sl], start=True, stop=True,
                                     perf_mode=DR)
                    if conv == "A":
                        nc.scalar.activation(out=qtM8[:, t, sl], in_=qp[:],
                                             func=AF.Identity,
                                             bias=u2x[:, t:t + 1], scale=1.0)
                    else:
                        nc.vector.tensor_scalar(out=qtM8[:, t, sl], in0=qp[:],
                                                scalar1=u2x[:, t:t + 1],
                                                scalar2=None, op0=OP.add)

            # qtM for pass 0 first (gates attention start); rest follow
            emit_qtm(0, QTE[0])

            def emit_exp(sT2, P2, sl2, eng):
                """exp over sT2[:, sl2, :] -> P2[:, sl2, :]; sl2 covers one
                or both pair halves (fused when both on one engine)."""
                if eng == "A":
                    nc.scalar.activation(out=P2[:, sl2, :],
                                         in_=sT2[:, sl2, :],
                                         func=AF.Exp, scale=SCALE)
                else:
                    nc.vector.tensor_scalar(
                        out=P2[:, sl2, :].bitcast(i8), in0=sT2[:, sl2, :],
                        scalar1=SCH_A, scalar2=SCH_B,
                        op0=OP.mult, op1=OP.add)

            def emit_pv_ones(h2p, rsp, pr, P2, npr):
                vpair = vTR[:, 256 * pr:256 * (pr + 1)].rearrange(
                    "p (two c) -> p two c", two=2)
                nc.tensor.matmul(h2p[:], vpair, P2[:], start=(pr == 0),
                                 stop=(pr == npr - 1), perf_mode=DR)
                nc.tensor.matmul(rsp[:], onesPv, P2[:], start=(pr == 0),
                                 stop=(pr == npr - 1), perf_mode=DR)

            def emit_finish(ic, h2p, rsp, split=False):
                """recip + normalize + out-proj + epilogue + DMA for pass ic.
                split=True pipelines four quarter-windows (tail latency),
                alternating the normalize between DVE and Pool."""
                HW = IW // 2 if split else IW
                for hw in range(IW // HW):
                    fl = slice(hw * HW, (hw + 1) * HW)
                    sl = slice(ic * IW + hw * HW, ic * IW + (hw + 1) * HW)
                    recipB = sp_.tile([C, HW], f32, tag="recipB",
                                      name=f"rcp{ic}_{hw}", bufs=2)
                    nc.vector.reciprocal_approx_fast(out=recipB[:],
                                                     in_=rsp[:, fl])
                    h2n = sp_.tile([C, HW], f32r, tag="h2n",
                                   name=f"h2n{ic}_{hw}", bufs=2)
                    nc.vector.tensor_tensor(out=h2n[:], in0=h2p[:, fl],
                                            in1=recipB[:], op=OP.mult)
                    pop = ring.tile([C, HW], f32, tag="sT",
                                    name=f"pop{ic}_{hw}")
                    nc.tensor.matmul(pop[:], woutR[:], h2n[:], start=True,
                                     stop=True)
                    outS = sp_.tile([C, HW], f32, tag="outS",
                                    name=f"outS{ic}_{hw}", bufs=2)
                    nc.vector.scalar_tensor_tensor(
                        out=outS[:], in0=pop[:], scalar=beffT[:],
                        in1=xS[:, sl], op0=OP.add, op1=OP.add)
                    nc.sync.dma_start(o_d[:, sl], outS[:])

            # ---- attention: flat loop over 4 passes x 32 j-tiles. The PV
            # lag and the per-pass finish both cross pass boundaries in
            # emission order, so PE's QK stream never drains at a boundary.
            vp = None
            vpbase = 0
            h2rs = {}
            pending_finish = None   # finish(ic) is emitted early in pass ic+1
            pend = []               # (ic, pr, P2) awaiting PV/ones, 2-late

            def get_h2rs(ic):
                if ic not in h2rs:
                    h2rs[ic] = (
                        ph2.tile([C, IW], f32, tag="h2", name=f"h2_{ic}"),
                        prs.tile([C, IW], f32, tag="rs", name=f"rs_{ic}"))
                return h2rs[ic]

            def flush_one():
                fic, pr, Pp = pend.pop(0)
                h2p, rsp = get_h2rs(fic)
                emit_pv_ones(h2p, rsp, pr, Pp, NJT // 2)

            for ic in range(NPASS):
                for pr in range(NJT // 2):
                    sT2 = ring.tile([C, 2, IW], f32, tag="sT",
                                    name=f"s{ic}_{pr}")
                    P2 = pP.tile([C, 2, IW], f8, tag="P",
                                 name=f"P{ic}_{pr}")
                    e0 = ASSIGN[NJT * ic + 2 * pr]
                    e1 = ASSIGN[NJT * ic + 2 * pr + 1]
                    for tp in range(2):
                        jt = 2 * pr + tp
                        with tc.high_priority(offset=24):
                            nc.tensor.matmul(
                                sT2[:, tp, :],
                                z2x[:, :, 128 * jt:128 * (jt + 1)],
                                qtM8[:, :, ic * IW:(ic + 1) * IW],
                                start=True, stop=True, perf_mode=DR)
                        if ic == 0:
                            # vT rides the same z-tile ldweights in pass 0
                            if jt % 8 == 0:
                                vp = ring.tile([C, 2 * IW], f32, tag="sT",
                                               name=f"vp{jt // 8}")
                                vpbase = jt // 8
                            nc.tensor.matmul(vp[:, 128 * (jt % 8):
                                                128 * (jt % 8 + 1)],
                                             z2x[:, :,
                                                 128 * jt:128 * (jt + 1)],
                                             wv2x8[:], start=True, stop=True,
                                             perf_mode=DR)
                            if jt % 8 == 7:
                                vce = VTE[vpbase % len(VTE)]
                                if vce == "A":
                                    nc.scalar.activation(
                                        out=vTR[:, 2 * IW * vpbase:
                                                2 * IW * (vpbase + 1)],
                                        in_=vp[:], func=AF.Identity,
                                        scale=1.0)
                                else:
                                    nc.vector.tensor_copy(
                                        vTR[:, 2 * IW * vpbase:
                                            2 * IW * (vpbase + 1)], vp[:])
                        if tp == 0 and e0 != e1:
                            emit_exp(sT2, P2, slice(0, 1), e0)
                    if e0 == e1:
                        emit_exp(sT2, P2, slice(0, 2), e0)  # fused pair op
                    else:
                        emit_exp(sT2, P2, slice(1, 2), e1)
                    pend.append((ic, pr, P2))
                    # PV/ones lag: pass ic's last pairs run into pass ic+1,
                    # giving the recip/normalize drain of h2/rs (bufs=1)
                    # half a pass of slack
                    lag = PVLAG if ic < NPASS - 1 else 3
                    while len(pend) > lag:
                        flush_one()
                    # previous pass's finish rides mid-pass so its out-proj
                    # matmul (gated by the recip/normalize chain) never
                    # blocks this pass's QK stream
                    if pr == 5 and pending_finish is not None:
                        emit_finish(*pending_finish)
                        pending_finish = None
                    # window ic+1's qtM rides mid-pass ic (needed only
                    # at pass ic+1), spreading conversion work across passes
                    if pr == 7 and ic < NPASS - 1:
                        emit_qtm(ic + 1, QTE[ic + 1])
                pending_finish = (ic,) + get_h2rs(ic)
            while pend:
                flush_one()
            emit_finish(*pending_finish, split=True)

    nc.compile()
    return nc


def host_inputs(x, gn_w, gn_b, w_qkv, b_qkv, w_out, b_out):
    """Build the 8 per-core input maps from the full problem inputs."""
    x = np.asarray(x, dtype=np.float32)
    B, _, N = x.shape
    S = N // 2
    w_qkv = np.asarray(w_qkv, np.float32)
    w_out = np.asarray(w_out, np.float32)
    b_qkv = np.asarray(b_qkv, np.float32)
    b_out = np.asarray(b_out, np.float32)
    gn_w = np.asarray(gn_w, np.float32)
    gn_b = np.asarray(gn_b, np.float32)

    M = (w_qkv[0:C].T @ w_qkv[C:2 * C]).astype(np.float32)   # [C, C]
    bqt = (w_qkv[C:2 * C].T @ b_qkv[0:C]).astype(np.float32)  # [C] k-side
    wv = w_qkv[2 * C:3 * C]                                   # [C, C]
    bv = b_qkv[2 * C:3 * C]
    perm = np.concatenate([np.arange(0, C, 2), np.arange(1, C, 2)])  # 2p+t
    gidx = np.arange(C) // GS
    gmask = (gidx[:, None] == gidx[None, :]).astype(np.float32) / GS

    b_eff = b_out + w_out @ bv
    bcat = np.stack([np.zeros(C, np.float32), np.zeros(C, np.float32),
                     b_eff, gn_w, gn_b], axis=1).astype(np.float32)
    # wcat: [Mc | wv^T | wout^T | gmask2x | gmask | bcat]
    wcat = np.concatenate(
        [M[:, perm], wv.T, w_out.T, gmask[:, perm], gmask, bcat],
        axis=1).astype(np.float32)
    # wcat2: [M2xc | wv2x | bcat2]; M2xc[p, t*C+o] = M[2p+t, perm(o)],
    # wv2x[p, t*C+cv] = wv[cv, 2p+t]
    M2xc = M[:, perm].reshape(NP, 2 * C)
    wv2x = wv.T.reshape(NP, 2 * C)
    bcat2 = np.concatenate([gn_w.reshape(NP, 2), bqt.reshape(NP, 2)],
                           axis=1).astype(np.float32)
    wcat2 = np.concatenate([M2xc, wv2x, bcat2], axis=1).astype(np.float32)

    in_maps = []
    for core in range(N_CORES):
        b, half = divmod(core, 2)
        xb = np.roll(x[b], -half * S, axis=1)
        in_maps.append({"x": np.ascontiguousarray(xb), "wcat": wcat,
                        "wcat2": wcat2})
    return in_maps


_NC_CACHE = {}
_RUNNER_CACHE = {}


def _make_runner(nc):
    """Compile-once runner: replicates bass2jax.run_bass_via_pjrt but keeps the
    jitted sharded callable so repeat executions skip recompilation."""
    import jax
    import concourse.mybir as mybir
    from jax.sharding import Mesh, PartitionSpec
    from jax.experimental.shard_map import shard_map
    from concourse.bass2jax import (_bass_exec_p, install_neuronx_cc_hook,
                                    partition_id_tensor)

    install_neuronx_cc_hook()
    partition_name = nc.partition_id_tensor.name if nc.partition_id_tensor else None
    in_names, out_names, out_avals, zero_shapes = [], [], [], []
    for alloc in nc.m.functions[0].allocations:
        if not isinstance(alloc, mybir.MemoryLocationSet):
            continue
        name = alloc.memorylocations[0].name
        if alloc.kind == "ExternalInput":
            if name == partition_name:
                continue
            in_names.append(name)
        elif alloc.kind == "ExternalOutput":
            out_names.append(name)
            shape = tuple(alloc.tensor_shape)
            dtype = mybir.dt.np(alloc.dtype)
            out_avals.append(jax.core.ShapedArray(shape, dtype))
            zero_shapes.append((shape, dtype))
    n_params = len(in_names)
    all_names = in_names + out_names
    if partition_name is not None:
        all_names = all_names + [partition_name]
    donate = tuple(range(n_params, n_params + len(out_names)))

    def _body(*args):
        operands = list(args)
        if partition_name is not None:
            operands.append(partition_id_tensor())
        return tuple(_bass_exec_p.bind(
            *operands, out_avals=tuple(out_avals), in_names=tuple(all_names),
            out_names=tuple(out_names), lowering_input_output_aliases=(),
            sim_require_finite=True, sim_require_nnan=True, nc=nc))

    devices = jax.devices()[:N_CORES]
    mesh = Mesh(np.asarray(devices), ("core",))
    specs = (PartitionSpec("core"),)
    sharded = jax.jit(
        shard_map(_body, mesh=mesh,
                  in_specs=specs * (n_params + len(out_names)),
                  out_specs=specs * len(out_names), check_rep=False),
        donate_argnums=donate, keep_unused=True)

    def run(in_maps):
        concat_in = [np.concatenate([np.asarray(m[nm]) for m in in_maps], axis=0)
                     for nm in in_names]
        concat_zeros = [np.zeros((N_CORES * s[0], *s[1:]), d) for s, d in zero_shapes]
        out_arrs = sharded(*concat_in, *concat_zeros)
        out_arrs = [np.asarray(a) for a in out_arrs]
        return [{nm: out_arrs[i].reshape(N_CORES, *out_avals[i].shape)[c]
                 for i, nm in enumerate(out_names)} for c in range(N_CORES)]

    return run


def get_runner(N=4096):
    if N not in _RUNNER_CACHE:
        if N not in _NC_CACHE:
            _NC_CACHE[N] = build(N)
        _RUNNER_CACHE[N] = _make_runner(_NC_CACHE[N])
    return _RUNNER_CACHE[N]


def kernel(x, gn_w, gn_b, w_qkv, b_qkv, w_out, b_out):
    from concourse._compat import axon_active

    x = np.asarray(x, dtype=np.float32)
    B, _, N = x.shape
    S = N // 2
    in_maps = host_inputs(x, gn_w, gn_b, w_qkv, b_qkv, w_out, b_out)
    if axon_active():
        results = get_runner(N)(in_maps)
    else:
        from concourse.bass_utils import run_bass_kernel_spmd

        if N not in _NC_CACHE:
            _NC_CACHE[N] = build(N)
        results = run_bass_kernel_spmd(_NC_CACHE[N], in_maps,
                                       core_ids=list(range(N_CORES))).results
    out = np.empty((B, C, N), dtype=np.float32)
    for core in range(N_CORES):
        b, half = divmod(core, 2)
        out[b, :, half * S:(half + 1) * S] = results[core]["out"]
    return out


# revision 10
# speedup vs baseline: 1.1954x; 1.0163x over previous
"""AttentionBlock (GroupNorm -> QKV -> full attention -> out-proj + residual)
for B=4, C=128, N=4096 on 8 Trainium2 NeuronCores.

Sharding: 8 cores = 4 batches x 2 query-slabs of N/2. Every core runs the
same program; the host rolls each core's x so its query slab is always
columns [0, N/2).

Key moves:
- All big matmuls are fp8 DoubleRow (0.5 cyc/col): channels split as
  c = 2p + t into a [64, 2, *] layout so the C=128 contraction rides the
  256-row DoubleRow path. z = a*x (GN scale only; the shift b folds into a
  per-channel bias u = M^T b + wk^T bq added during the qtM psum->fp8
  conversion, since scores s[j,i] = z_j . (M^T z_i + u)).
- exp is split across the two PSUM-capable engines: ACT runs real Exp,
  DVE runs a Schraudolph exp (int8(A*s + B) bitcast as fp8e4m3, one
  tensor_scalar op). GPSIMD cannot touch PSUM (and has no TensorScalar),
  so it only carries SBUF-side const conversions. Scores for a j-tile
  pair land in one [C, 2, 512] PSUM tile; a same-engine pair is one
  fused 1024-wide exp op. P pairs feed fp8 DoubleRow PV and rowsum
  (ones) matmuls directly - no DVE pair-adds anywhere.
- One PSUM layout for the whole kernel: a 3-slot ring of 2-bank pair
  tiles (scores, qtM/vT staging, out-proj) + 1 bank each for the PV and
  rowsum accumulators. No mid-kernel pool transitions.
- v comes from z via wv fp8 DoubleRow sharing QK's ldweights in pass 0;
  GroupNorm stats run once in 128-layout (exact b) with a dup'd Newton
  chain in the split layout for a; psum->fp8 conversions are spread
  A/D by a tuned schedule; qtM windows 2..4 ride earlier passes.
- Per-pass (i-window 512) finish: recip + normalize + epilogue (DVE),
  out-proj (PE), DMA out - emitted mid-next-pass so PE's QK stream never
  stalls at a pass boundary; PV/ones run 6 pairs late for the same
  reason. The last pass's finish is split in halves with an ACT assist.
End-to-end relative error vs the fp32 reference is ~6.4e-4
(fp8-dominated); TimelineSim per-core time ~72.9us (baseline was 94.0us).
"""

import math
import sys

if "/opt/trn_rl_repo" not in sys.path:
    sys.path.insert(0, "/opt/trn_rl_repo")

import numpy as np

C = 128
G = 8
GS = C // G  # channels per group
EPS = 1e-5
N_CORES = 8
NP = C // 2  # 64: partition count of the split-channel layout


def build(N=4096, repeat=1, cfg=None):
    """Build the per-core Bass program. Returns the compiled Bacc module."""
    cfg = dict(cfg or {})
    P0 = cfg.get("p0", (9, 7))           # pass-0 exp PAIR counts A/D
    PS = cfg.get("ps", (10, 6))           # steady-pass exp PAIR counts
    VTE = cfg.get("vte", "AADA")     # vT conv engines by chunk
    QTE = cfg.get("qte", "AADD")         # qtM conv engines by window
    PVLAG = cfg.get("pvlag", 6)          # pairs of PV/ones lag
    FINPR = cfg.get("finpr", 11)          # pair at which prev finish emits
    import concourse.bacc as bacc
    import concourse.bass as bass
    import concourse.mybir as mybir
    import concourse.tile as tile

    f32 = mybir.dt.float32
    f32r = mybir.dt.float32r
    f8 = mybir.dt.float8e4
    i8 = mybir.dt.int8
    i32 = mybir.dt.int32
    AF = mybir.ActivationFunctionType
    OP = mybir.AluOpType
    DR = mybir.MatmulPerfMode.DoubleRow

    S = N // 2            # query slab width per core
    IW = 512              # i-window per pass
    NPASS = S // IW       # 4
    NJT = N // 128        # 32 j tiles
    XC = 512              # xS DMA/bnstats chunk
    NXC = N // XC         # 8
    X2C = 1024            # x2x DMA / z2x chunk
    NX2 = N // X2C        # 4
    SCALE = 1.0 / math.sqrt(C)
    # Schraudolph: int8 y = trunc(A*s_raw + B) bitcast fp8e4m3 ~ exp(SCALE*s)
    SCH_A = 8.0 * math.log2(math.e) * SCALE
    SCH_B = 8.0 * (7.0 - 0.045) + 0.5

    # exp engine per (pass, jtile): A=ACT, D=DVE, P=Pool, rate-weighted
    # (ACT 0.61us, DVE 0.66, Pool 0.81 per tile) with D/P's extra per-pass
    # work (recip/normalize on D, epilogue on P, conversions in pass 0)
    # subtracted from their shares.
    def mk_assign(na, nd):
        # per-PAIR engine letters, doubled to tiles: same-engine pairs let
        # one amortized pair-op cover both halves
        out, acc = [], {"A": 0.0, "D": 0.0}
        want = {"A": na, "D": nd}
        for i in range(NJT // 2):
            e = max(want, key=lambda k: want[k] * (i + 1) / (NJT // 2) - acc[k])
            acc[e] += 1
            out += [e, e]
        return out

    ASSIGN = mk_assign(*P0)
    for _ in range(NPASS - 1):
        ASSIGN += mk_assign(*PS)

    nc = bacc.Bacc("TRN2", target_bir_lowering=False, debug=False)

    x_d = nc.dram_tensor("x", [C, N], f32, kind="ExternalInput").ap()
    # wcat cols: [Mc | wv^T | wout^T | gmask2x | gmask | bcat(5)]
    w_d = nc.dram_tensor("wcat", [C, 5 * C + 5], f32, kind="ExternalInput").ap()
    # wcat2 cols: [M2xc | wv2x | bcat2(4)]
    w2_d = nc.dram_tensor("wcat2", [NP, 4 * C + 4], f32,
                          kind="ExternalInput").ap()
    o_d = nc.dram_tensor("out", [C, S], f32, kind="ExternalOutput").ap()

    with tile.TileContext(nc) as tc:
        with tc.tile_pool(name="consts", bufs=1) as cp, \
             tc.tile_pool(name="big", bufs=1) as bp, \
             tc.tile_pool(name="small", bufs=3) as sp_, \
             tc.tile_pool(name="pP", bufs=10) as pP, \
             tc.tile_pool(name="ring", bufs=3, space="PSUM") as ring, \
             tc.tile_pool(name="ph2", bufs=1, space="PSUM") as ph2, \
             tc.tile_pool(name="prs", bufs=1, space="PSUM") as prs:
            # ---- DMAs: xS chunks first (stats chain), consts, x2x ----
            xS = bp.tile([C, N], f32, tag="x")
            for c in range(NXC):
                nc.sync.dma_start(xS[:, c * XC:(c + 1) * XC],
                                  x_d[:, c * XC:(c + 1) * XC])
            wS = cp.tile([C, 5 * C + 5], f32, tag="w")
            nc.sync.dma_start(wS[:], w_d[:])
            bS = wS[:, 5 * C:5 * C + 5]
            x2x = bp.tile([NP, 2, N], f32, tag="x2x")
            x2v = x_d.rearrange("(p two) n -> p two n", two=2)
            nc.sync.dma_start(x2x[:, :, 0:X2C], x2v[:, :, 0:X2C])
            wc2 = cp.tile([NP, 4 * C + 4], f32, tag="w2")
            nc.sync.dma_start(wc2[:], w2_d[:])
            b2S = wc2[:, 4 * C:4 * C + 4]
            for c in range(1, NX2):
                nc.sync.dma_start(x2x[:, :, c * X2C:(c + 1) * X2C],
                                  x2v[:, :, c * X2C:(c + 1) * X2C])

            # ---- ACT exp-table preload (runs during DMA) ----
            trash = sp_.tile([C, 1], f32, tag="trash")
            nc.vector.memset(trash[:], 0.0)
            nc.scalar.activation(out=trash[:], in_=trash[:], func=AF.Exp,
                                 scale=1.0)

            # ---- const conversions ----
            M2xc8 = cp.tile([NP, 2, C], f8, tag="m2xc8")
            nc.gpsimd.tensor_copy(M2xc8[:], wc2[:, 0:2 * C])
            wv2x8 = cp.tile([NP, 2, C], f8, tag="wv2x8")
            nc.gpsimd.tensor_copy(wv2x8[:], wc2[:, 2 * C:4 * C])
            onesP = cp.tile([C, 2 * C], f8, tag="onesp")
            nc.gpsimd.memset(onesP[:], 1.0)
            onesPv = onesP[:].rearrange("p (two c) -> p two c", two=2)
            woutR = cp.tile([C, C], f32r, tag="woutr")
            nc.gpsimd.tensor_copy(woutR[:], wS[:, 2 * C:3 * C])

            # persistent big tensors
            z2x = bp.tile([NP, 2, N], f8, tag="z2x")
            qtM8 = bp.tile([NP, 2, S], f8, tag="qtm8")
            vTR = bp.tile([C, N], f8, tag="vT")

            def rsqrt_chain(eng, shape, gmean, gex2, tag):
                """Magic-Newton rsqrt(var+eps) on [part, k] APs -> inv tile."""
                part, k = shape
                gm2 = sp_.tile([part, k], f32, tag=tag + "gm2")
                eng.tensor_tensor(out=gm2[:], in0=gmean, in1=gmean, op=OP.mult)
                xv = sp_.tile([part, k], f32, tag=tag + "xv")
                eng.tensor_tensor(out=xv[:], in0=gex2, in1=gm2[:],
                                  op=OP.subtract)
                eng.tensor_scalar(out=xv[:], in0=xv[:], scalar1=1.0,
                                  scalar2=EPS, op0=OP.mult, op1=OP.add)
                yh = sp_.tile([part, k], i32, tag=tag + "yh")
                eng.tensor_scalar(out=yh[:], in0=xv[:].bitcast(i32),
                                  scalar1=1, scalar2=None,
                                  op0=OP.logical_shift_right)
                eng.tensor_scalar(out=yh[:], in0=yh[:], scalar1=-1,
                                  scalar2=0x5F3759DF, op0=OP.mult, op1=OP.add)
                y0 = yh[:].bitcast(f32)
                tN = sp_.tile([part, k], f32, tag=tag + "tN")
                eng.tensor_tensor(out=tN[:], in0=y0, in1=y0, op=OP.mult)
                eng.tensor_tensor(out=tN[:], in0=tN[:], in1=xv[:], op=OP.mult)
                eng.tensor_scalar(out=tN[:], in0=tN[:], scalar1=-0.5,
                                  scalar2=1.5, op0=OP.mult, op1=OP.add)
                inv = sp_.tile([part, k], f32, tag=tag + "inv")
                eng.tensor_tensor(out=inv[:], in0=y0, in1=tN[:], op=OP.mult)
                return inv

            # ---- GroupNorm stats: bnstats (DVE) -> group matmuls (PE) ----
            st6 = sp_.tile([C, NXC, 6], f32, tag="st6")
            for c in range(NXC):
                nc.vector.bn_stats(out=st6[:, c, :],
                                   in_=xS[:, c * XC:(c + 1) * XC])
            mv = sp_.tile([C, 2], f32, tag="mv")
            nc.vector.bn_aggr(out=mv[:], in_=st6[:])
            nc.vector.scalar_tensor_tensor(out=mv[:, 1:2], in0=mv[:, 0:1],
                                           scalar=mv[:, 0:1], in1=mv[:, 1:2],
                                           op0=OP.mult, op1=OP.add)
            gps = ring.tile([C, 2], f32, tag="sT", name="gps")
            nc.tensor.matmul(gps[:], wS[:, 4 * C:5 * C], mv[:], start=True,
                             stop=True)
            gap = ring.tile([NP, 2, 2], f32, tag="sT", name="gap")
            for t in range(2):
                nc.tensor.matmul(gap[:, t, :],
                                 wS[:, 3 * C + NP * t:3 * C + NP * (t + 1)],
                                 mv[:], start=True, stop=True)

            # split layout scale a2x first (it gates z2x); single copies
            # out of PSUM (ALU ops may read at most one PSUM operand)
            gas = sp_.tile([NP, 2, 2], f32, tag="gas")
            nc.vector.tensor_copy(gas[:], gap[:])
            inv2x = rsqrt_chain(nc.vector, [NP, 2], gas[:, :, 0:1],
                                gas[:, :, 1:2], "n2")
            a2x = sp_.tile([NP, 2], f32, tag="a2x")
            nc.vector.tensor_tensor(out=a2x[:], in0=b2S[:, 0:2], in1=inv2x[:],
                                    op=OP.mult)
            gst = sp_.tile([C, 2], f32, tag="gst")
            nc.vector.tensor_copy(gst[:], gps[:])
            inv128 = rsqrt_chain(nc.vector, [C, 1], gst[:, 0:1], gst[:, 1:2],
                                 "n1")
            aT = sp_.tile([C, 1], f32, tag="aT")
            nc.vector.tensor_tensor(out=aT[:], in0=bS[:, 3:4], in1=inv128[:],
                                    op=OP.mult)
            bT = sp_.tile([C, 1], f32, tag="bT")
            nc.vector.tensor_tensor(out=bT[:], in0=gst[:, 0:1], in1=aT[:],
                                    op=OP.mult)
            nc.vector.tensor_tensor(out=bT[:], in0=bS[:, 4:5], in1=bT[:],
                                    op=OP.subtract)

            # u2x = (M^T b + bqt) in split layout; beff = bcat + w_out wv^T b
            u_p = ring.tile([NP, 2], f32, tag="sT", name="u_p")
            for t in range(2):
                nc.tensor.matmul(u_p[:, t:t + 1], wS[:, NP * t:NP * (t + 1)],
                                 bT[:], start=True, stop=True)
            u2x = sp_.tile([NP, 2], f32, tag="u2xf")
            nc.vector.tensor_tensor(out=u2x[:], in0=u_p[:], in1=b2S[:, 2:4],
                                    op=OP.add)
            vc_p = ring.tile([C, 2], f32, tag="sT", name="vc_p")
            nc.tensor.matmul(vc_p[:, 0:1], wS[:, C:2 * C], bT[:], start=True,
                             stop=True)
            vcS = sp_.tile([C, 1], f32, tag="vcS")
            nc.vector.tensor_copy(vcS[:], vc_p[:, 0:1])
            wov_p = ring.tile([C, 2], f32, tag="sT", name="wov_p")
            nc.tensor.matmul(wov_p[:, 0:1], wS[:, 2 * C:3 * C], vcS[:],
                             start=True, stop=True)
            beffT = sp_.tile([C, 1], f32, tag="beffT")
            nc.vector.tensor_tensor(out=beffT[:], in0=bS[:, 2:3],
                                    in1=wov_p[:, 0:1], op=OP.add)

            # ---- z2x = a2x * x2x -> fp8 (all on DVE: 2x-port mode there).
            # high_priority so the scheduler never defers z chunks behind
            # pass-0 exps (z gates every QK of the j-tile it covers).
            with tc.high_priority():
                for c in range(NX2):
                    sl = slice(c * X2C, (c + 1) * X2C)
                    for t in range(2):
                        nc.vector.tensor_scalar(out=z2x[:, t, sl],
                                                in0=x2x[:, t, sl],
                                                scalar1=a2x[:, t:t + 1],
                                                scalar2=None, op0=OP.mult)

            def emit_qtm(ic, conv):
                """qtM for i-window ic: 2 DR matmuls + 2 psum->fp8(+u) convs.
                conv in {'D','P','A'}."""
                sl = slice(ic * IW, (ic + 1) * IW)
                for t in range(2):
                    qp = ring.tile([NP, IW], f32, tag="sT",
                                   name=f"qp{ic}_{t}")
                    nc.tensor.matmul(qp[:], M2xc8[:, :, NP * t:NP * (t + 1)],
                                     z2x[:, :, sl], start=True, stop=True,
                                     perf_mode=DR)
                    if conv == "A":
                        nc.scalar.activation(out=qtM8[:, t, sl], in_=qp[:],
                                             func=AF.Identity,
                                             bias=u2x[:, t:t + 1], scale=1.0)
                    else:
                        nc.vector.tensor_scalar(out=qtM8[:, t, sl], in0=qp[:],
                                                scalar1=u2x[:, t:t + 1],
                                                scalar2=None, op0=OP.add)

            # qtM for pass 0 first (gates attention start); rest follow
            emit_qtm(0, QTE[0])

            def emit_exp(sT2, P2, sl2, eng):
                """exp over sT2[:, sl2, :] -> P2[:, sl2, :]; sl2 covers one
                or both pair halves (fused when both on one engine)."""
                if eng == "A":
                    nc.scalar.activation(out=P2[:, sl2, :],
                                         in_=sT2[:, sl2, :],
                                         func=AF.Exp, scale=SCALE)
                else:
                    nc.vector.tensor_scalar(
                        out=P2[:, sl2, :].bitcast(i8), in0=sT2[:, sl2, :],
                        scalar1=SCH_A, scalar2=SCH_B,
                        op0=OP.mult, op1=OP.add)

            def emit_pv_ones(h2p, rsp, pr, P2, npr):
                vpair = vTR[:, 256 * pr:256 * (pr + 1)].rearrange(
                    "p (two c) -> p two c", two=2)
                nc.tensor.matmul(h2p[:], vpair, P2[:], start=(pr == 0),
                                 stop=(pr == npr - 1), perf_mode=DR)
                nc.tensor.matmul(rsp[:], onesPv, P2[:], start=(pr == 0),
                                 stop=(pr == npr - 1), perf_mode=DR)

            def emit_finish(ic, h2p, rsp, split=False):
                """recip + normalize + out-proj + epilogue + DMA for pass ic.
                split=True pipelines four quarter-windows (tail latency),
                alternating the normalize between DVE and Pool."""
                HW = IW // 2 if split else IW
                for hw in range(IW // HW):
                    fl = slice(hw * HW, (hw + 1) * HW)
                    sl = slice(ic * IW + hw * HW, ic * IW + (hw + 1) * HW)
                    recipB = sp_.tile([C, HW], f32, tag="recipB",
                                      name=f"rcp{ic}_{hw}", bufs=2)
                    nc.vector.reciprocal_approx_fast(out=recipB[:],
                                                     in_=rsp[:, fl])
                    h2n = sp_.tile([C, HW], f32r, tag="h2n",
                                   name=f"h2n{ic}_{hw}", bufs=2)
                    nc.vector.tensor_tensor(out=h2n[:], in0=h2p[:, fl],
                                            in1=recipB[:], op=OP.mult)
                    pop = ring.tile([C, HW], f32, tag="sT",
                                    name=f"pop{ic}_{hw}")
                    nc.tensor.matmul(pop[:], woutR[:], h2n[:], start=True,
                                     stop=True)
                    outS = sp_.tile([C, HW], f32, tag="outS",
                                    name=f"outS{ic}_{hw}", bufs=2)
                    nc.vector.scalar_tensor_tensor(
                        out=outS[:], in0=pop[:], scalar=beffT[:],
                        in1=xS[:, sl], op0=OP.add, op1=OP.add)
                    nc.sync.dma_start(o_d[:, sl], outS[:])

            # ---- attention: flat loop over 4 passes x 32 j-tiles. The PV
            # lag and the per-pass finish both cross pass boundaries in
            # emission order, so PE's QK stream never drains at a boundary.
            vp = None
            vpbase = 0
            h2rs = {}
            pending_finish = None   # finish(ic) is emitted early in pass ic+1
            pend = []               # (ic, pr, P2) awaiting PV/ones, 2-late

            def get_h2rs(ic):
                if ic not in h2rs:
                    h2rs[ic] = (
                        ph2.tile([C, IW], f32, tag="h2", name=f"h2_{ic}"),
                        prs.tile([C, IW], f32, tag="rs", name=f"rs_{ic}"))
                return h2rs[ic]

            def flush_one():
                fic, pr, Pp = pend.pop(0)
                h2p, rsp = get_h2rs(fic)
                emit_pv_ones(h2p, rsp, pr, Pp, NJT // 2)

            for ic in range(NPASS):
                for pr in range(NJT // 2):
                    sT2 = ring.tile([C, 2, IW], f32, tag="sT",
                                    name=f"s{ic}_{pr}")
                    P2 = pP.tile([C, 2, IW], f8, tag="P",
                                 name=f"P{ic}_{pr}")
                    e0 = ASSIGN[NJT * ic + 2 * pr]
                    e1 = ASSIGN[NJT * ic + 2 * pr + 1]
                    for tp in range(2):
                        jt = 2 * pr + tp
                        with tc.high_priority(offset=24):
                            nc.tensor.matmul(
                                sT2[:, tp, :],
                                z2x[:, :, 128 * jt:128 * (jt + 1)],
                                qtM8[:, :, ic * IW:(ic + 1) * IW],
                                start=True, stop=True, perf_mode=DR)
                        if ic == 0:
                            # vT rides the same z-tile ldweights in pass 0
                            if jt % 8 == 0:
                                vp = ring.tile([C, 2 * IW], f32, tag="sT",
                                               name=f"vp{jt // 8}")
                                vpbase = jt // 8
                            nc.tensor.matmul(vp[:, 128 * (jt % 8):
                                                128 * (jt % 8 + 1)],
                                             z2x[:, :,
                                                 128 * jt:128 * (jt + 1)],
                                             wv2x8[:], start=True, stop=True,
                                             perf_mode=DR)
                            if jt % 8 == 7:
                                vce = VTE[vpbase % len(VTE)]
                                if vce == "A":
                                    nc.scalar.activation(
                                        out=vTR[:, 2 * IW * vpbase:
                                                2 * IW * (vpbase + 1)],
                                        in_=vp[:], func=AF.Identity,
                                        scale=1.0)
                                else:
                                    nc.vector.tensor_copy(
                                        vTR[:, 2 * IW * vpbase:
                                            2 * IW * (vpbase + 1)], vp[:])
                        if tp == 0 and e0 != e1:
                            emit_exp(sT2, P2, slice(0, 1), e0)
                    if e0 == e1:
                        emit_exp(sT2, P2, slice(0, 2), e0)  # fused pair op
                    else:
                        emit_exp(sT2, P2, slice(1, 2), e1)
                    pend.append((ic, pr, P2))
                    # PV/ones lag: pass ic's last pairs run into pass ic+1,
                    # giving the recip/normalize drain of h2/rs (bufs=1)
                    # half a pass of slack
                    lag = PVLAG if ic < NPASS - 1 else 3
                    while len(pend) > lag:
                        flush_one()
                    # previous pass's finish rides mid-pass so its out-proj
                    # matmul (gated by the recip/normalize chain) never
                    # blocks this pass's QK stream
                    if pr == FINPR and pending_finish is not None:
                        emit_finish(*pending_finish)
                        pending_finish = None
                    # window ic+1's qtM rides mid-pass ic (needed only
                    # at pass ic+1), spreading conversion work across passes
                    if pr == 7 and ic < NPASS - 1:
                        emit_qtm(ic + 1, QTE[ic + 1])
                pending_finish = (ic,) + get_h2rs(ic)
            while pend:
                flush_one()
            emit_finish(*pending_finish, split=True)

    nc.compile()
    return nc


def host_inputs(x, gn_w, gn_b, w_qkv, b_qkv, w_out, b_out):
    """Build the 8 per-core input maps from the full problem inputs."""
    x = np.asarray(x, dtype=np.float32)
    B, _, N = x.shape
    S = N // 2
    w_qkv = np.asarray(w_qkv, np.float32)
    w_out = np.asarray(w_out, np.float32)
    b_qkv = np.asarray(b_qkv, np.float32)
    b_out = np.asarray(b_out, np.float32)
    gn_w = np.asarray(gn_w, np.float32)
    gn_b = np.asarray(gn_b, np.float32)

    M = (w_qkv[0:C].T @ w_qkv[C:2 * C]).astype(np.float32)   # [C, C]
    bqt = (w_qkv[C:2 * C].T @ b_qkv[0:C]).astype(np.float32)  # [C] k-side
    wv = w_qkv[2 * C:3 * C]                                   # [C, C]
    bv = b_qkv[2 * C:3 * C]
    perm = np.concatenate([np.arange(0, C, 2), np.arange(1, C, 2)])  # 2p+t
    gidx = np.arange(C) // GS
    gmask = (gidx[:, None] == gidx[None, :]).astype(np.float32) / GS

    b_eff = b_out + w_out @ bv
    bcat = np.stack([np.zeros(C, np.float32), np.zeros(C, np.float32),
                     b_eff, gn_w, gn_b], axis=1).astype(np.float32)
    # wcat: [Mc | wv^T | wout^T | gmask2x | gmask | bcat]
    wcat = np.concatenate(
        [M[:, perm], wv.T, w_out.T, gmask[:, perm], gmask, bcat],
        axis=1).astype(np.float32)
    # wcat2: [M2xc | wv2x | bcat2]; M2xc[p, t*C+o] = M[2p+t, perm(o)],
    # wv2x[p, t*C+cv] = wv[cv, 2p+t]
    M2xc = M[:, perm].reshape(NP, 2 * C)
    wv2x = wv.T.reshape(NP, 2 * C)
    bcat2 = np.concatenate([gn_w.reshape(NP, 2), bqt.reshape(NP, 2)],
                           axis=1).astype(np.float32)
    wcat2 = np.concatenate([M2xc, wv2x, bcat2], axis=1).astype(np.float32)

    in_maps = []
    for core in range(N_CORES):
        b, half = divmod(core, 2)
        xb = np.roll(x[b], -half * S, axis=1)
        in_maps.append({"x": np.ascontiguousarray(xb), "wcat": wcat,
                        "wcat2": wcat2})
    return in_maps


_NC_CACHE = {}
_RUNNER_CACHE = {}


def _make_runner(nc):
    """Compile-once runner: replicates bass2jax.run_bass_via_pjrt but keeps the
    jitted sharded callable so repeat executions skip recompilation."""
    import jax
    import concourse.mybir as mybir
    from jax.sharding import Mesh, PartitionSpec
    from jax.experimental.shard_map import shard_map
    from concourse.bass2jax import (_bass_exec_p, install_neuronx_cc_hook,
                                    partition_id_tensor)

    install_neuronx_cc_hook()
    partition_name = nc.partition_id_tensor.name if nc.partition_id_tensor else None
    in_names, out_names, out_avals, zero_shapes = [], [], [], []
    for alloc in nc.m.functions[0].allocations:
        if not isinstance(alloc, mybir.MemoryLocationSet):
            continue
        name = alloc.memorylocations[0].name
        if alloc.kind == "ExternalInput":
            if name == partition_name:
                continue
            in_names.append(name)
        elif alloc.kind == "ExternalOutput":
            out_names.append(name)
            shape = tuple(alloc.tensor_shape)
            dtype = mybir.dt.np(alloc.dtype)
            out_avals.append(jax.core.ShapedArray(shape, dtype))
            zero_shapes.append((shape, dtype))
    n_params = len(in_names)
    all_names = in_names + out_names
    if partition_name is not None:
        all_names = all_names + [partition_name]
    donate = tuple(range(n_params, n_params + len(out_names)))

    def _body(*args):
        operands = list(args)
        if partition_name is not None:
            operands.append(partition_id_tensor())
        return tuple(_bass_exec_p.bind(
            *operands, out_avals=tuple(out_avals), in_names=tuple(all_names),
            out_names=tuple(out_names), lowering_input_output_aliases=(),
            sim_require_finite=True, sim_require_nnan=True, nc=nc))

    devices = jax.devices()[:N_CORES]
    mesh = Mesh(np.asarray(devices), ("core",))
    specs = (PartitionSpec("core"),)
    sharded = jax.jit(
        shard_map(_body, mesh=mesh,
                  in_specs=specs * (n_params + len(out_names)),
                  out_specs=specs * len(out_names), check_rep=False),
        donate_argnums=donate, keep_unused=True)

    def run(in_maps):
        concat_in = [np.concatenate([np.asarray(m[nm]) for m in in_maps], axis=0)
                     for nm in in_names]
        concat_zeros = [np.zeros((N_CORES * s[0], *s[1:]), d) for s, d in zero_shapes]
        out_arrs = sharded(*concat_in, *concat_zeros)
        out_arrs = [np.asarray(a) for a in out_arrs]
        return [{nm: out_arrs[i].reshape(N_CORES, *out_avals[i].shape)[c]
                 for i, nm in enumerate(out_names)} for c in range(N_CORES)]

    return run


def get_runner(N=4096):
    if N not in _RUNNER_CACHE:
        if N not in _NC_CACHE:
            _NC_CACHE[N] = build(N)
        _RUNNER_CACHE[N] = _make_runner(_NC_CACHE[N])
    return _RUNNER_CACHE[N]


def kernel(x, gn_w, gn_b, w_qkv, b_qkv, w_out, b_out):
    from concourse._compat import axon_active

    x = np.asarray(x, dtype=np.float32)
    B, _, N = x.shape
    S = N // 2
    in_maps = host_inputs(x, gn_w, gn_b, w_qkv, b_qkv, w_out, b_out)
    if axon_active():
        results = get_runner(N)(in_maps)
    else:
        from concourse.bass_utils import run_bass_kernel_spmd

        if N not in _NC_CACHE:
            _NC_CACHE[N] = build(N)
        results = run_bass_kernel_spmd(_NC_CACHE[N], in_maps,
                                       core_ids=list(range(N_CORES))).results
    out = np.empty((B, C, N), dtype=np.float32)
    for core in range(N_CORES):
        b, half = divmod(core, 2)
        out[b, :, half * S:(half + 1) * S] = results[core]["out"]
    return out
